# revision 10
# baseline (speedup 1.0000x reference)
"""CapsuleNet forward kernel for 8 Trainium2 NeuronCores (pure data parallel).

Host side: im2col + weight-layout prep in numpy; batch 512 sharded 64/core.
Device side (per core), v3 design:
  - y stored P-inner [128, kt, 20, 20, P] so prim-conv rhs slices
    [6, 6, P] have a contiguous inner dim -> full-rate PE streaming
    (216ns/504-col matmul vs 302ns for the P-outer layout)
  - conv0 computed pass-wide from (pos, image)-interleaved im2col
    columns; PSUM evacuates with contiguous writes into the y layout
  - prim PSUM is (pos-major, image-minor) [128, 36, P]; evacuated by 4
    ACT copies straight into capsule-major u[q, j, b, k]
  - routing chain for overlapped passes runs on DVE + ACT + GPSIMD
    (partition sums via gpsimd.partition_all_reduce) so the PE streams
    uninterrupted; the LAST pass's chain instead uses PE ones-matmul
    partition+j reduction and gpsimd broadcast to cut the exposed tail
  - head: pass-0 xcols DMA'd before the 10.6MB resident weights
"""

import sys

if "/opt/trn_rl_repo" not in sys.path:
    sys.path.insert(0, "/opt/trn_rl_repo")

from contextlib import ExitStack

import ml_dtypes
import numpy as np

import concourse.bacc as bacc
import concourse.bass as bass
import concourse.bass_isa as bass_isa
import concourse.tile as tile
from concourse import mybir

F32 = mybir.dt.float32
BF16 = mybir.dt.bfloat16
AF = mybir.ActivationFunctionType
OP = mybir.AluOpType
RED = bass_isa.ReduceOp

N_CORES = 8
B_FULL = 512
B_CORE = B_FULL // N_CORES

PASSES = (14, 14, 14, 14, 8)
CHUNK = 512  # conv0 psum chunk columns


def build(B=B_CORE, passes=None, loops=1):
    """Build the Bass module for one core processing B images.

    loops>1 repeats the whole program (same output) for benchmarking."""
    if passes is None:
        passes = PASSES if B == B_CORE else None
    if passes is None:
        out, rem = [], B
        while rem > 0:
            p = min(14, rem)
            out.append(p)
            rem -= p
        passes = tuple(out)
    assert sum(passes) == B
    assert all(p <= 14 for p in passes)

    nc = bacc.Bacc("TRN2")

    # ---- DRAM I/O ----
    xcols_d = nc.dram_tensor("xcols", [81, 400 * B], BF16,
                             kind="ExternalInput")
    c0wT_d = nc.dram_tensor("c0wT", [81, 256], BF16, kind="ExternalInput")
    c0b_d = nc.dram_tensor("c0b2", [128, 2], F32, kind="ExternalInput")
    pbr_d = nc.dram_tensor("pbr", [128, 2], F32, kind="ExternalInput")
    # resident prim weights: [p(=ci%128), tap, kt(=ci//128), r(=co%2), q(=co//2)]
    wres_d = nc.dram_tensor("wres", [128, 81, 2, 2, 128], BF16,
                            kind="ExternalInput")
    # capsule weights: [m(=i//9), j(=i%9), o, k]
    dwr_d = nc.dram_tensor("dwr", [128, 9, 10, 8], F32, kind="ExternalInput")
    onesb_d = nc.dram_tensor("onesb", [128, 1], BF16, kind="ExternalInput")
    onesr_d = nc.dram_tensor("onesr", [1, 128], BF16, kind="ExternalInput")
    out_d = nc.dram_tensor("out", [B, 10], F32, kind="ExternalOutput")

    with ExitStack() as ctx:
        tc = ctx.enter_context(tile.TileContext(nc))

        consts = ctx.enter_context(tc.tile_pool(name="consts", bufs=1))
        xcp = ctx.enter_context(tc.tile_pool(name="xcp", bufs=2))
        yp = ctx.enter_context(tc.tile_pool(name="yp", bufs=2))
        up = ctx.enter_context(tc.tile_pool(name="up", bufs=2))
        xhp = ctx.enter_context(tc.tile_pool(name="xhp", bufs=2))
        tmpp = ctx.enter_context(tc.tile_pool(name="tmpp", bufs=1))
        smp = ctx.enter_context(tc.tile_pool(name="smp", bufs=1))
        sch = ctx.enter_context(tc.tile_pool(name="sch", bufs=1))
        pc0 = ctx.enter_context(tc.tile_pool(name="pc0", bufs=3, space="PSUM"))
        ppr = ctx.enter_context(tc.tile_pool(name="ppr", bufs=2, space="PSUM"))
        pch = ctx.enter_context(tc.tile_pool(name="pch", bufs=1, space="PSUM"))

        offs = []
        b0 = 0
        for P in passes:
            offs.append(b0)
            b0 += P

        # ---- head-critical DMAs first: conv0 needs c0wT + early xcols ----
        c0wT_t = consts.tile([81, 256], BF16)
        nc.sync.dma_start(out=c0wT_t, in_=c0wT_d[:, :])
        # pass-0 xcols in two segments so the first conv0 chunks don't wait
        # for the full 900KB transfer
        XC0A = 2 * CHUNK
        xc0a_t = xcp.tile([81, XC0A], BF16, tag="xca", name="xc0a")
        nc.sync.dma_start(out=xc0a_t, in_=xcols_d[:, 0:XC0A])
        c0b_t = consts.tile([128, 2], F32)
        nc.sync.dma_start(out=c0b_t, in_=c0b_d[:, :])
        xc0b_t = xcp.tile([81, 400 * passes[0] - XC0A], BF16, tag="xcb",
                          name="xc0b")
        nc.sync.dma_start(out=xc0b_t,
                          in_=xcols_d[:, XC0A:400 * passes[0]])

        pb_t = consts.tile([128, 2], F32)
        nc.sync.dma_start(out=pb_t, in_=pbr_d[:, :])
        dwr_t = consts.tile([128, 9, 10, 8], F32)
        nc.sync.dma_start(out=dwr_t, in_=dwr_d[:, :, :, :])
        onesb_t = consts.tile([128, 1], BF16)
        nc.sync.dma_start(out=onesb_t, in_=onesb_d[:, :])
        onesr_t = consts.tile([1, 128], BF16)
        nc.sync.dma_start(out=onesr_t, in_=onesr_d[:, :])
        dwrb_t = consts.tile([128, 9, 10, 8], BF16)
        nc.vector.tensor_copy(out=dwrb_t[:], in_=dwr_t[:])

        def xcols_load(i):
            P = passes[i]
            xc_t = xcp.tile([81, 400 * P], BF16, tag="xc", name="xc_t")
            o = offs[i] * 400
            nc.sync.dma_start(out=xc_t, in_=xcols_d[:, o:o + 400 * P])
            return xc_t

        _CB = {"n": 0}

        def conv0_ops(y_t, xc_t, P, segs=None):
            """Pass-wide conv0: (pos,img)-interleaved columns; returns list
            of callbacks each doing one chunk matmul+relu-evac. segs: list
            of (tile, col_start, col_end) overriding the single xc_t."""
            total = 400 * P
            yf = y_t.rearrange("p t h w b -> p t (h w b)")
            if segs is None:
                segs = [(xc_t, 0, total)]

            def rhs_for(c0, cw):
                for t_, s_, e_ in segs:
                    if c0 >= s_ and c0 + cw <= e_:
                        return t_[:, c0 - s_:c0 - s_ + cw]
                raise AssertionError("chunk crosses segment boundary")

            cbs = []
            for mt in range(2):
                for c0 in range(0, total, CHUNK):
                    cw = min(CHUNK, total - c0)

                    def cb(mt=mt, c0=c0, cw=cw):
                        ps = pc0.tile([128, CHUNK], F32, tag="pc0",
                                      name="c0ps")
                        nc.tensor.matmul(
                            out=ps[:, 0:cw],
                            lhsT=c0wT_t[:, mt * 128:(mt + 1) * 128],
                            rhs=rhs_for(c0, cw),
                            start=True, stop=True,
                        )
                        nc.scalar.activation(
                            out=yf[:, mt, c0:c0 + cw], in_=ps[:, 0:cw],
                            func=AF.Relu, bias=c0b_t[:, mt:mt + 1],
                            scale=1.0,
                        )
                    cbs.append(cb)
            return cbs

        wres_ts = []

        def load_wres():
            # per-chunk weight tiles so early prim taps don't wait on the
            # full 10.6MB resident load
            for t0 in range(0, 81, 9):
                wt = consts.tile([128, 9, 2, 2, 128], BF16,
                                 name=f"wres{t0}")
                nc.sync.dma_start(out=wt, in_=wres_d[:, t0:t0 + 9])
                wres_ts.append(wt)

        def prim_pass(y_t, P, interleave=None):
            """prim caps conv: accumulate 81 taps x 2kt into 2 r-PSUM tiles
            shaped [128, 6, 6, P] (pos-major, image-minor)."""
            pr = [ppr.tile([128, 6, 6, P], F32, tag=f"ppr{r}",
                           name=f"ppr_{r}") for r in range(2)]
            for t in range(81):
                if interleave and t in interleave:
                    for cb in interleave[t]:
                        cb()
                kh, kw = t // 9, t % 9
                wt = wres_ts[t // 9]
                for kt in range(2):
                    for r in range(2):
                        rhs = y_t[:, kt, kh:kh + 12:2, kw:kw + 12:2, :]
                        nc.tensor.matmul(
                            out=pr[r].rearrange("p h w b -> p (h w b)"),
                            lhsT=wt[:, t % 9, kt, r, :],
                            rhs=rhs,
                            start=(t == 0 and kt == 0),
                            stop=(t == 80 and kt == 1),
                        )
            return pr

        def stage2_pass(pr, P):
            """evacuate prim PSUM (+bias) into capsule-major u[q, j, b, k].
            PSUM columns are (pos, b); 36r+pos = 8j+k."""
            u_t = up.tile([128, 9, P, 8], F32, tag="u", name="u_t")
            V0 = pr[0].rearrange("p h w b -> p (h w) b")
            V1 = pr[1].rearrange("p h w b -> p (h w) b")
            nc.scalar.activation(
                out=u_t[:, 0:4, :, :],
                in_=V0[:, 0:32, :].rearrange("p (j k) b -> p j k b", k=8)
                .transpose([0, 1, 3, 2]),
                func=AF.Identity, bias=pb_t[:, 0:1], scale=1.0,
            )
            nc.scalar.activation(
                out=u_t[:, 4, :, 0:4],
                in_=V0[:, 32:36, :].transpose([0, 2, 1]),
                func=AF.Identity, bias=pb_t[:, 0:1], scale=1.0,
            )
            nc.scalar.activation(
                out=u_t[:, 4, :, 4:8],
                in_=V1[:, 0:4, :].transpose([0, 2, 1]),
                func=AF.Identity, bias=pb_t[:, 1:2], scale=1.0,
            )
            nc.scalar.activation(
                out=u_t[:, 5:9, :, :],
                in_=V1[:, 4:36, :].rearrange("p (j k) b -> p j k b", k=8)
                .transpose([0, 1, 3, 2]),
                func=AF.Identity, bias=pb_t[:, 1:2], scale=1.0,
            )
            return u_t

        def vrow(S_t, alpha, tag, P, np_=128):
            """squash scalar per (b,o): v = s^3/((1+s^2)(|s|+eps)), s=alpha*S.
            Works on [np_, P, 10] tiles (np_=128 replicated, or 1)."""
            sh = [np_, P, 10]
            pool = smp if np_ == 128 else sch
            if alpha != 1.0:
                ts_ = pool.tile(sh, F32, tag="vr_ts", name="vr_ts")
                nc.scalar.activation(out=ts_[:], in_=S_t[:], func=AF.Copy,
                                     scale=alpha)
            else:
                ts_ = S_t
            s2 = pool.tile(sh, F32, tag="vr_s2", name="vr_s2")
            nc.scalar.activation(out=s2[:], in_=S_t[:], func=AF.Square,
                                 scale=alpha)
            ab = pool.tile(sh, F32, tag="vr_ab", name="vr_ab")
            nc.scalar.activation(out=ab[:], in_=S_t[:], func=AF.Abs,
                                 scale=alpha)
            nc.vector.scalar_tensor_tensor(
                out=ab[:], in0=s2[:], scalar=1.0, in1=ab[:],
                op0=OP.add, op1=OP.mult,
            )
            nc.vector.tensor_scalar_add(ab[:], ab[:], 1e-8)
            nc.vector.reciprocal(out=ab[:], in_=ab[:])
            nc.vector.tensor_mul(s2[:], ts_[:], s2[:])
            v = pool.tile(sh, F32, tag=tag, name=tag)
            nc.vector.tensor_mul(v[:], s2[:], ab[:])
            return v

        def squash_u(u_t, P):
            """in-place squash of u over k plus bf16 copy + x_hat."""
            usq = tmpp.tile([128, 9, P, 8], F32, tag="usq", name="usq")
            nc.scalar.activation(out=usq[:], in_=u_t[:], func=AF.Square)
            n2 = smp.tile([128, 9, P], F32, tag="n2", name="n2")
            nc.vector.tensor_reduce(out=n2[:], in_=usq[:],
                                    axis=mybir.AxisListType.X, op=OP.add)
            nrm = smp.tile([128, 9, P], F32, tag="nrm", name="nrm")
            nc.scalar.activation(out=nrm[:], in_=n2[:], func=AF.Sqrt)
            nc.vector.tensor_scalar_add(n2[:], n2[:], 1.0)
            nc.vector.reciprocal(out=n2[:], in_=n2[:])
            nc.vector.tensor_mul(nrm[:], nrm[:], n2[:])
            nc.vector.tensor_mul(
                u_t[:], u_t[:],
                nrm.unsqueeze(3).broadcast_to([128, 9, P, 8]),
            )
            ub = tmpp.tile([128, 9, P, 8], BF16, tag="ub", name="ub")
            nc.vector.tensor_copy(out=ub[:], in_=u_t[:])

            X_t = xhp.tile([128, 9, P, 10], BF16, tag="X", name="X_t")
            with nc.allow_low_precision(reason="bf16 x_hat (8-term dots)"):
                for o in range(10):
                    xt = tmpp.tile([128, 9, P, 8], BF16, tag="xh",
                                   name="xh_tmp")
                    nc.vector.tensor_mul(
                        xt[:], ub[:],
                        dwrb_t[:, :, o, :].unsqueeze(2)
                        .broadcast_to([128, 9, P, 8]),
                    )
                    nc.vector.tensor_reduce(out=X_t[:, :, :, o], in_=xt[:],
                                            axis=mybir.AxisListType.X,
                                            op=OP.add)
            return X_t

        def pe_sum(src_t, P, name):
            """S[1, P*10] = sum over partitions and j of src[128, 9, P, 10]
            via 9 accumulating ones-matmuls; evac to a [1, P, 10] f32 tile."""
            ps = pch.tile([1, P * 10], F32, tag="pchS", name=name + "p")
            for j in range(9):
                nc.tensor.matmul(
                    out=ps[:, :],
                    lhsT=onesb_t[:, :],
                    rhs=src_t[:, j].rearrange("p b o -> p (b o)"),
                    start=(j == 0), stop=(j == 8),
                )
            Sr = sch.tile([1, P, 10], F32, tag="Sr_" + name, name=name)
            nc.scalar.activation(out=Sr.rearrange("p b o -> p (b o)"),
                                 in_=ps[:, :], func=AF.Identity, scale=1.0)
            return Sr

        def bcast_row(vrow_t, P, name):
            """broadcast [1, P, 10] f32 -> [128, P, 10] bf16 via gpsimd."""
            vb = sch.tile([1, P, 10], BF16, tag="vb_" + name, name="vb")
            nc.vector.tensor_copy(out=vb[:], in_=vrow_t[:])
            wb = smp.tile([128, P, 10], BF16, tag="wbt", name="wb_" + name)
            nc.gpsimd.partition_broadcast(wb[:], vb[:], 128)
            return wb

        def chain_sched(u_t, P, b0, sched=None):
            """squash -> x_hat -> 3 routing iterations -> |v| -> out DMA.
            Partition(+j) sums run on PE via ones-matmuls; vrow on a single
            partition; gpsimd only broadcasts. When `sched` (an interleave
            dict for the NEXT pass's prim, plus tap positions) is given, the
            S/iter stages are deferred so the PE-stream ops land at taps
            where their DVE inputs are surely ready; else emitted inline."""
            X_t = squash_u(u_t, P)
            st = {}

            def s0_cb():
                S0 = pe_sum(X_t, P, "S0")
                st["w"] = vrow(S0, 0.1, "w_acc_t", P, np_=1)
                st["wb"] = bcast_row(st["w"], P, "i1")

            def iter_cb(it):
                final = it == 2

                def cb():
                    wb = st["wb"]
                    L = tmpp.tile([128, 9, P, 10], BF16, tag="L", name="Lt")
                    with nc.allow_low_precision(reason="bf16 routing logits"):
                        nc.vector.tensor_mul(
                            L[:], X_t[:],
                            wb.unsqueeze(1).broadcast_to([128, 9, P, 10]),
                        )
                    nc.scalar.activation(out=L[:], in_=L[:], func=AF.Exp)
                    Z = smp.tile([128, 9, P], F32, tag="Z", name="Zt")
                    nc.vector.tensor_reduce(out=Z[:], in_=L[:],
                                            axis=mybir.AxisListType.X,
                                            op=OP.add)
                    rZ = smp.tile([128, 9, P], BF16, tag="rZ", name="rZt")
                    with nc.allow_low_precision(reason="bf16 softmax denom"):
                        nc.vector.reciprocal(out=rZ[:], in_=Z[:])
                        nc.vector.tensor_mul(L[:], L[:], X_t[:])
                        nc.vector.tensor_mul(
                            L[:], L[:],
                            rZ.unsqueeze(3).broadcast_to([128, 9, P, 10]),
                        )
                    S = pe_sum(L, P, f"S{it}")
                    if not final:
                        v = vrow(S, 1.0, "v1_t", P, np_=1)
                        nc.vector.tensor_add(st["w"][:], st["w"][:], v[:])
                        st["wb"] = bcast_row(st["w"], P, "i2")
                    else:
                        v = vrow(S, 1.0, "v2_t", P, np_=1)
                        fo = sch.tile([1, P, 10], F32, tag="fo_t", name="fo")
                        nc.scalar.activation(out=fo[:], in_=v[:], func=AF.Abs)
                        nc.sync.dma_start(
                            out=out_d[b0:b0 + P, :],
                            in_=fo[0:1, :, :],
                        )
                return cb

            if sched is None:
                s0_cb()
                iter_cb(1)()
                iter_cb(2)()
            else:
                inter, taps = sched
                inter.setdefault(taps[0], []).append(s0_cb)
                inter.setdefault(taps[1], []).append(iter_cb(1))
                inter.setdefault(taps[2], []).append(iter_cb(2))

        for _loop in range(loops):
            y_cur = yp.tile([128, 2, 20, 20, passes[0]], BF16, tag="y",
                            name="y_t")
            if _loop == 0:
                # head: xc0a/xc0b segments already DMA'd above, then the
                # big resident load
                load_wres()
                cbs0 = conv0_ops(y_cur, None, passes[0],
                                 segs=[(xc0a_t, 0, XC0A),
                                       (xc0b_t, XC0A, 400 * passes[0])])
            else:
                xc_cur = xcols_load(0)
                cbs0 = conv0_ops(y_cur, xc_cur, passes[0])
            for cb in cbs0:
                cb()

            inter = {}
            for i, P in enumerate(passes):
                nxt = {}
                y_next = None
                if i + 1 < len(passes):
                    Pn = passes[i + 1]
                    y_next = yp.tile([128, 2, 20, 20, Pn], BF16, tag="y",
                                     name="y_t")
                    xc_box = []

                    def load_next(i=i):
                        xc_box.append(xcols_load(i + 1))

                    inter.setdefault(4, []).append(load_next)
                    cbs_box = []

                    def make_cbs(y_next=y_next, Pn=Pn):
                        cbs_box.extend(conv0_ops(y_next, xc_box[0], Pn))

                    inter.setdefault(24, []).append(make_cbs)
                    # spread conv0 chunks over taps 25..72
                    n_cb = 2 * ((400 * Pn + CHUNK - 1) // CHUNK)
                    for ci in range(n_cb):
                        def run_cb(ci=ci):
                            cbs_box[ci]()
                        inter.setdefault(25 + 2 * ci, []).append(run_cb)
                pr = prim_pass(y_cur, P, interleave=inter)
                u_t = stage2_pass(pr, P)
                if i + 1 < len(passes):
                    # defer the chain's PE-stream stages into the next
                    # pass's prim at taps where their inputs are ready
                    taps = (36, 54, 72) if passes[i + 1] == 14 else (56, 70, 79)
                    chain_sched(u_t, P, offs[i], sched=(nxt, taps))
                else:
                    chain_sched(u_t, P, offs[i], sched=None)
                inter = nxt
                y_cur = y_next

    nc.compile()
    return nc


# ---------------- host side ----------------

_CACHE = {}


def _prep(x, conv0_w, conv0_b, prim_w, prim_b, digit_w):
    B = x.shape[0]
    xw = np.lib.stride_tricks.sliding_window_view(x[:, 0], (9, 9), axis=(1, 2))
    # (B, 20, 20, 9, 9) -> (B, 9, 9, 20, 20) -> (B, 81, 400)
    xcols_std = np.ascontiguousarray(
        xw.transpose(0, 3, 4, 1, 2).reshape(B, 81, 400)
    )
    c0wT = np.ascontiguousarray(
        conv0_w.reshape(256, 81).T
    ).astype(ml_dtypes.bfloat16)
    c0b2 = np.ascontiguousarray(
        conv0_b.reshape(2, 128).T, dtype=np.float32
    )
    # prim weights resident layout [p, t, kt, r, q]: co = 2q+r, ci = kt*128+p
    pw = prim_w.reshape(128, 2, 2, 128, 81)  # (q, r, kt, p, t)
    wres = np.ascontiguousarray(
        pw.transpose(3, 4, 2, 1, 0)  # (p, t, kt, r, q)
    ).astype(ml_dtypes.bfloat16)
    pbr = np.ascontiguousarray(prim_b.reshape(128, 2), dtype=np.float32)
    dwr = np.ascontiguousarray(
        digit_w[:, :, 0, :].transpose(1, 0, 2).reshape(128, 9, 10, 8),
        dtype=np.float32,
    )
    return xcols_std, c0wT, c0b2, pbr, wres, dwr


def _xcols_interleave(xcols_core):
    """[Bc, 81, 400] -> [81, 400*Bc] with per-pass (pos, image) interleave."""
    Bc = xcols_core.shape[0]
    passes = PASSES if Bc == B_CORE else None
    if passes is None:
        out, rem = [], Bc
        while rem > 0:
            p = min(14, rem)
            out.append(p)
            rem -= p
        passes = tuple(out)
    blocks = []
    b0 = 0
    for P in passes:
        blk = xcols_core[b0:b0 + P]              # [P, 81, 400]
        blocks.append(blk.transpose(1, 2, 0).reshape(81, 400 * P))
        b0 += P
    return np.ascontiguousarray(
        np.concatenate(blocks, axis=1)
    ).astype(ml_dtypes.bfloat16)


def make_in_maps(x, conv0_w, conv0_b, prim_w, prim_b, digit_w):
    x = np.asarray(x, dtype=np.float32)
    conv0_w = np.asarray(conv0_w, dtype=np.float32)
    conv0_b = np.asarray(conv0_b, dtype=np.float32)
    prim_w = np.asarray(prim_w, dtype=np.float32)
    prim_b = np.asarray(prim_b, dtype=np.float32)
    digit_w = np.asarray(digit_w, dtype=np.float32)

    xcols_std, c0wT, c0b2, pbr, wres, dwr = _prep(
        x, conv0_w, conv0_b, prim_w, prim_b, digit_w
    )
    onesb = np.ones((128, 1), dtype=ml_dtypes.bfloat16)
    onesr = np.ones((1, 128), dtype=ml_dtypes.bfloat16)
    in_maps = []
    for c in range(N_CORES):
        sl = slice(c * B_CORE, (c + 1) * B_CORE)
        in_maps.append(
            {
                "xcols": _xcols_interleave(xcols_std[sl]),
                "c0wT": c0wT,
                "c0b2": c0b2,
                "pbr": pbr,
                "wres": wres,
                "dwr": dwr,
                "onesb": onesb,
                "onesr": onesr,
            }
        )
    return in_maps


def kernel(x, conv0_w, conv0_b, prim_w, prim_b, digit_w):
    from concourse.bass_utils import run_bass_kernel_spmd

    in_maps = make_in_maps(x, conv0_w, conv0_b, prim_w, prim_b, digit_w)

    if "nc" not in _CACHE:
        _CACHE["nc"] = build(B_CORE)
    nc = _CACHE["nc"]

    res = run_bass_kernel_spmd(nc, in_maps, core_ids=list(range(N_CORES)))
    out = np.concatenate([r["out"] for r in res.results], axis=0)
    return out.astype(np.float32)


if __name__ == "__main__":
    # quick smoke build
    nc = build()
    print("build ok")


# revision 23
# speedup vs baseline: 1.2073x; 1.2073x over previous
"""CapsuleNet forward kernel for 8 Trainium2 NeuronCores (pure data parallel).

Host side: im2col + weight-layout prep in numpy; batch 512 sharded 64/core.
Device side (per core), v3 design:
  - y stored P-inner [128, kt, 20, 20, P] so prim-conv rhs slices
    [6, 6, P] have a contiguous inner dim -> full-rate PE streaming
    (216ns/504-col matmul vs 302ns for the P-outer layout)
  - conv0 computed pass-wide from (pos, image)-interleaved im2col
    columns; PSUM evacuates with contiguous writes into the y layout
  - prim PSUM is (pos-major, image-minor) [128, 36, P]; evacuated by 4
    ACT copies straight into capsule-major u[q, j, b, k]
  - routing chain for overlapped passes runs on DVE + ACT + GPSIMD
    (partition sums via gpsimd.partition_all_reduce) so the PE streams
    uninterrupted; the LAST pass's chain instead uses PE ones-matmul
    partition+j reduction and gpsimd broadcast to cut the exposed tail
  - head: pass-0 xcols DMA'd before the 10.6MB resident weights
"""

import sys

if "/opt/trn_rl_repo" not in sys.path:
    sys.path.insert(0, "/opt/trn_rl_repo")

from contextlib import ExitStack

import ml_dtypes
import numpy as np

import concourse.bacc as bacc
import concourse.bass as bass
import concourse.bass_isa as bass_isa
import concourse.tile as tile
from concourse import mybir

F32 = mybir.dt.float32
BF16 = mybir.dt.bfloat16
AF = mybir.ActivationFunctionType
OP = mybir.AluOpType
RED = bass_isa.ReduceOp

N_CORES = 8
B_FULL = 512
B_CORE = B_FULL // N_CORES

PASSES = (14, 14, 14, 14, 8)
CHUNK = 512  # conv0 psum chunk columns


def build(B=B_CORE, passes=None, loops=1):
    """Build the Bass module for one core processing B images.

    loops>1 repeats the whole program (same output) for benchmarking."""
    if passes is None:
        passes = PASSES if B == B_CORE else None
    if passes is None:
        out, rem = [], B
        while rem > 0:
            p = min(14, rem)
            out.append(p)
            rem -= p
        passes = tuple(out)
    assert sum(passes) == B
    assert all(p <= 14 for p in passes)

    nc = bacc.Bacc("TRN2")

    # ---- DRAM I/O ----
    xcols_d = nc.dram_tensor("xcols", [81, 400 * B], BF16,
                             kind="ExternalInput")
    c0wT_d = nc.dram_tensor("c0wT", [81, 256], BF16, kind="ExternalInput")
    c0b_d = nc.dram_tensor("c0b2", [128, 2], F32, kind="ExternalInput")
    pbr_d = nc.dram_tensor("pbr", [128, 2], F32, kind="ExternalInput")
    # resident prim weights: [p(=ci%128), tap, kt(=ci//128), r(=co%2), q(=co//2)]
    wres_d = nc.dram_tensor("wres", [128, 81, 2, 2, 128], BF16,
                            kind="ExternalInput")
    # capsule weights: [m(=i//9), j(=i%9), o, k]
    dwr_d = nc.dram_tensor("dwr", [128, 9, 10, 8], F32, kind="ExternalInput")
    onesb_d = nc.dram_tensor("onesb", [128, 1], BF16, kind="ExternalInput")
    onesr_d = nc.dram_tensor("onesr", [1, 128], BF16, kind="ExternalInput")
    out_d = nc.dram_tensor("out", [B, 10], F32, kind="ExternalOutput")

    with ExitStack() as ctx:
        tc = ctx.enter_context(tile.TileContext(nc))

        consts = ctx.enter_context(tc.tile_pool(name="consts", bufs=1))
        xcp = ctx.enter_context(tc.tile_pool(name="xcp", bufs=2))
        yp = ctx.enter_context(tc.tile_pool(name="yp", bufs=2))
        up = ctx.enter_context(tc.tile_pool(name="up", bufs=2))
        xhp = ctx.enter_context(tc.tile_pool(name="xhp", bufs=2))
        tmpp = ctx.enter_context(tc.tile_pool(name="tmpp", bufs=1))
        smp = ctx.enter_context(tc.tile_pool(name="smp", bufs=1))
        sch = ctx.enter_context(tc.tile_pool(name="sch", bufs=1))
        pc0 = ctx.enter_context(tc.tile_pool(name="pc0", bufs=3, space="PSUM"))
        ppr = ctx.enter_context(tc.tile_pool(name="ppr", bufs=2, space="PSUM"))
        pch = ctx.enter_context(tc.tile_pool(name="pch", bufs=1, space="PSUM"))

        offs = []
        b0 = 0
        for P in passes:
            offs.append(b0)
            b0 += P

        # ---- head-critical DMAs first: conv0 needs c0wT + early xcols ----
        c0wT_t = consts.tile([81, 256], BF16)
        nc.sync.dma_start(out=c0wT_t, in_=c0wT_d[:, :])
        xc0_t = xcp.tile([81, 400 * passes[0]], BF16, tag="xc", name="xc_t")
        nc.sync.dma_start(out=xc0_t, in_=xcols_d[:, 0:400 * passes[0]])
        c0b_t = consts.tile([128, 2], F32)
        nc.sync.dma_start(out=c0b_t, in_=c0b_d[:, :])
        pb_t = consts.tile([128, 2], F32)
        nc.sync.dma_start(out=pb_t, in_=pbr_d[:, :])
        dwr_t = consts.tile([128, 9, 10, 8], F32)
        nc.sync.dma_start(out=dwr_t, in_=dwr_d[:, :, :, :])
        onesb_t = consts.tile([128, 1], BF16)
        nc.sync.dma_start(out=onesb_t, in_=onesb_d[:, :])
        onesr_t = consts.tile([1, 128], BF16)
        nc.sync.dma_start(out=onesr_t, in_=onesr_d[:, :])
        dwrb_t = consts.tile([128, 9, 10, 8], BF16)
        nc.vector.tensor_copy(out=dwrb_t[:], in_=dwr_t[:])

        def xcols_load(i):
            P = passes[i]
            xc_t = xcp.tile([81, 400 * P], BF16, tag="xc", name="xc_t")
            o = offs[i] * 400
            nc.sync.dma_start(out=xc_t, in_=xcols_d[:, o:o + 400 * P])
            return xc_t

        _CB = {"n": 0}

        def conv0_ops(y_t, xc_t, P, segs=None):
            """Pass-wide conv0: (pos,img)-interleaved columns; returns list
            of callbacks each doing one chunk matmul+relu-evac. segs: list
            of (tile, col_start, col_end) overriding the single xc_t."""
            total = 400 * P
            yf = y_t.rearrange("p t h w b -> p t (h w b)")
            if segs is None:
                segs = [(xc_t, 0, total)]

            def rhs_for(c0, cw):
                for t_, s_, e_ in segs:
                    if c0 >= s_ and c0 + cw <= e_:
                        return t_[:, c0 - s_:c0 - s_ + cw]
                raise AssertionError("chunk crosses segment boundary")

            cbs = []
            for mt in range(2):
                for c0 in range(0, total, CHUNK):
                    cw = min(CHUNK, total - c0)

                    def cb(mt=mt, c0=c0, cw=cw):
                        ps = pc0.tile([128, CHUNK], F32, tag="pc0",
                                      name="c0ps")
                        nc.tensor.matmul(
                            out=ps[:, 0:cw],
                            lhsT=c0wT_t[:, mt * 128:(mt + 1) * 128],
                            rhs=rhs_for(c0, cw),
                            start=True, stop=True,
                        )
                        nc.scalar.activation(
                            out=yf[:, mt, c0:c0 + cw], in_=ps[:, 0:cw],
                            func=AF.Relu, bias=c0b_t[:, mt:mt + 1],
                            scale=1.0,
                        )
                    cbs.append(cb)
            return cbs

        wres_ts = []

        def load_wres():
            # per-chunk weight tiles so early prim taps don't wait on the
            # full 10.6MB resident load
            for t0 in range(0, 81, 9):
                wt = consts.tile([128, 9, 2, 2, 128], BF16,
                                 name=f"wres{t0}")
                nc.sync.dma_start(out=wt, in_=wres_d[:, t0:t0 + 9])
                wres_ts.append(wt)

        def prim_pass(y_t, P, interleave=None):
            """prim caps conv: accumulate 81 taps x 2kt into 2 r-PSUM tiles
            shaped [128, 6, 6, P] (pos-major, image-minor)."""
            pr = [ppr.tile([128, 6, 6, P], F32, tag=f"ppr{r}",
                           name=f"ppr_{r}") for r in range(2)]
            for t in range(81):
                if interleave and t in interleave:
                    for cb in interleave[t]:
                        cb()
                kh, kw = t // 9, t % 9
                wt = wres_ts[t // 9]
                for kt in range(2):
                    for r in range(2):
                        rhs = y_t[:, kt, kh:kh + 12:2, kw:kw + 12:2, :]
                        nc.tensor.matmul(
                            out=pr[r].rearrange("p h w b -> p (h w b)"),
                            lhsT=wt[:, t % 9, kt, r, :],
                            rhs=rhs,
                            start=(t == 0 and kt == 0),
                            stop=(t == 80 and kt == 1),
                        )
            return pr

        def stage2_pass(pr, P):
            """evacuate prim PSUM (+bias) into capsule-major u[q, j, b, k].
            PSUM columns are (pos, b); 36r+pos = 8j+k."""
            u_t = up.tile([128, 9, P, 8], F32, tag="u", name="u_t")
            V0 = pr[0].rearrange("p h w b -> p (h w) b")
            V1 = pr[1].rearrange("p h w b -> p (h w) b")
            nc.scalar.activation(
                out=u_t[:, 0:4, :, :],
                in_=V0[:, 0:32, :].rearrange("p (j k) b -> p j k b", k=8)
                .transpose([0, 1, 3, 2]),
                func=AF.Identity, bias=pb_t[:, 0:1], scale=1.0,
            )
            nc.scalar.activation(
                out=u_t[:, 4, :, 0:4],
                in_=V0[:, 32:36, :].transpose([0, 2, 1]),
                func=AF.Identity, bias=pb_t[:, 0:1], scale=1.0,
            )
            nc.scalar.activation(
                out=u_t[:, 4, :, 4:8],
                in_=V1[:, 0:4, :].transpose([0, 2, 1]),
                func=AF.Identity, bias=pb_t[:, 1:2], scale=1.0,
            )
            nc.scalar.activation(
                out=u_t[:, 5:9, :, :],
                in_=V1[:, 4:36, :].rearrange("p (j k) b -> p j k b", k=8)
                .transpose([0, 1, 3, 2]),
                func=AF.Identity, bias=pb_t[:, 1:2], scale=1.0,
            )
            return u_t

        def vrow(S_t, alpha, tag, P, np_=128):
            """squash scalar per (b,o): v = s^3/((1+s^2)(|s|+eps)), s=alpha*S.
            Works on [np_, P, 10] tiles (np_=128 replicated, or 1)."""
            sh = [np_, P, 10]
            pool = smp if np_ == 128 else sch
            if alpha != 1.0:
                ts_ = pool.tile(sh, F32, tag="vr_ts", name="vr_ts")
                nc.scalar.activation(out=ts_[:], in_=S_t[:], func=AF.Copy,
                                     scale=alpha)
            else:
                ts_ = S_t
            s2 = pool.tile(sh, F32, tag="vr_s2", name="vr_s2")
            nc.scalar.activation(out=s2[:], in_=S_t[:], func=AF.Square,
                                 scale=alpha)
            ab = pool.tile(sh, F32, tag="vr_ab", name="vr_ab")
            nc.scalar.activation(out=ab[:], in_=S_t[:], func=AF.Abs,
                                 scale=alpha)
            nc.vector.scalar_tensor_tensor(
                out=ab[:], in0=s2[:], scalar=1.0, in1=ab[:],
                op0=OP.add, op1=OP.mult,
            )
            nc.vector.tensor_scalar_add(ab[:], ab[:], 1e-8)
            nc.vector.reciprocal_approx_fast(out=ab[:], in_=ab[:])
            nc.vector.tensor_mul(s2[:], ts_[:], s2[:])
            v = pool.tile(sh, F32, tag=tag, name=tag)
            nc.vector.tensor_mul(v[:], s2[:], ab[:])
            return v

        def squash_u(u_t, P):
            """in-place squash of u over k plus bf16 copy + x_hat."""
            usq = tmpp.tile([128, 9, P, 8], F32, tag="usq", name="usq")
            nc.scalar.activation(out=usq[:], in_=u_t[:], func=AF.Square)
            n2 = smp.tile([128, 9, P], F32, tag="n2", name="n2")
            nc.vector.tensor_reduce(out=n2[:], in_=usq[:],
                                    axis=mybir.AxisListType.X, op=OP.add)
            nrm = smp.tile([128, 9, P], F32, tag="nrm", name="nrm")
            nc.scalar.activation(out=nrm[:], in_=n2[:], func=AF.Sqrt)
            nc.vector.tensor_scalar_add(n2[:], n2[:], 1.0)
            nc.vector.reciprocal_approx_fast(out=n2[:], in_=n2[:])
            nc.vector.tensor_mul(nrm[:], nrm[:], n2[:])
            ub = tmpp.tile([128, 9, P, 8], BF16, tag="ub", name="ub")
            with nc.allow_low_precision(reason="bf16 squashed u"):
                nc.vector.tensor_mul(
                    ub[:], u_t[:],
                    nrm.unsqueeze(3).broadcast_to([128, 9, P, 8]),
                )

            X_t = xhp.tile([128, 9, P, 10], BF16, tag="X", name="X_t")
            with nc.allow_low_precision(reason="bf16 x_hat (8-term dots)"):
                for o in range(10):
                    xt = tmpp.tile([128, 9, P, 8], BF16, tag="xh",
                                   name="xh_tmp")
                    nc.vector.tensor_mul(
                        xt[:], ub[:],
                        dwrb_t[:, :, o, :].unsqueeze(2)
                        .broadcast_to([128, 9, P, 8]),
                    )
                    nc.vector.tensor_reduce(out=X_t[:, :, :, o], in_=xt[:],
                                            axis=mybir.AxisListType.X,
                                            op=OP.add)
            return X_t

        def pe_sum(src_t, P, name, scale01=False):
            """S[1, P*10] = sum over partitions and j of src[128, 9, P, 10]
            via 9 accumulating ones-matmuls; evac to a [1, P, 10] f32 tile.
            scale01: use the 0.1-valued lhsT column (uniform softmax S0)."""
            col = 1 if scale01 else 0
            ps = pch.tile([1, P * 10], F32, tag="pchS", name=name + "p")
            for j in range(9):
                nc.tensor.matmul(
                    out=ps[:, :],
                    lhsT=onesb_t[:, col:col + 1],
                    rhs=src_t[:, j].rearrange("p b o -> p (b o)"),
                    start=(j == 0), stop=(j == 8),
                )
            Sr = sch.tile([1, P, 10], F32, tag="Sr_" + name, name=name)
            nc.scalar.activation(out=Sr.rearrange("p b o -> p (b o)"),
                                 in_=ps[:, :], func=AF.Identity, scale=1.0)
            return Sr

        def bcast_row(vrow_t, P, name):
            """broadcast [1, P, 10] f32 -> [128, P, 10] bf16 via an
            outer-product ones-matmul (PE) + ACT evac."""
            vb = sch.tile([1, P, 10], BF16, tag="vb_" + name, name="vb")
            nc.vector.tensor_copy(out=vb[:], in_=vrow_t[:])
            ps = pch.tile([128, P * 10], F32, tag="pchS", name="bc_" + name)
            nc.tensor.matmul(
                out=ps[:, :],
                lhsT=onesr_t[:, :],
                rhs=vb.rearrange("p b o -> p (b o)"),
                start=True, stop=True,
            )
            wb = smp.tile([128, P, 10], BF16, tag="wbt", name="wb_" + name)
            nc.scalar.activation(out=wb.rearrange("p b o -> p (b o)"),
                                 in_=ps[:, :], func=AF.Identity, scale=1.0)
            return wb

        def chain_sched(u_t, P, b0, sched=None):
            """squash -> x_hat -> 3 routing iterations -> |v| -> out DMA.
            Partition(+j) sums run on PE via ones-matmuls; vrow on a single
            partition; gpsimd only broadcasts. When `sched` (an interleave
            dict for the NEXT pass's prim, plus tap positions) is given, the
            S/iter stages are deferred so the PE-stream ops land at taps
            where their DVE inputs are surely ready; else emitted inline."""
            X_t = squash_u(u_t, P)
            st = {}

            def s0_cb():
                S0 = pe_sum(X_t, P, "S0")
                st["w"] = vrow(S0, 0.1, "w_acc_t", P, np_=1)
                st["wb"] = bcast_row(st["w"], P, "i1")

            def iter_cb(it):
                final = it == 2

                def cb():
                    wb = st["wb"]
                    L = tmpp.tile([128, 9, P, 10], BF16, tag="L", name="Lt")
                    with nc.allow_low_precision(reason="bf16 routing logits"):
                        nc.vector.tensor_mul(
                            L[:], X_t[:],
                            wb.unsqueeze(1).broadcast_to([128, 9, P, 10]),
                        )
                    nc.scalar.activation(out=L[:], in_=L[:], func=AF.Exp)
                    Z = smp.tile([128, 9, P], F32, tag="Z", name="Zt")
                    nc.vector.tensor_reduce(out=Z[:], in_=L[:],
                                            axis=mybir.AxisListType.X,
                                            op=OP.add)
                    rZf = smp.tile([128, 9, P], F32, tag="rZf", name="rZt")
                    nc.vector.reciprocal_approx_fast(out=rZf[:], in_=Z[:])
                    rZ = smp.tile([128, 9, P], BF16, tag="rZ", name="rZb")
                    with nc.allow_low_precision(reason="bf16 softmax denom"):
                        nc.vector.tensor_copy(out=rZ[:], in_=rZf[:])
                        nc.vector.tensor_mul(L[:], L[:], X_t[:])
                        nc.vector.tensor_mul(
                            L[:], L[:],
                            rZ.unsqueeze(3).broadcast_to([128, 9, P, 10]),
                        )
                    S = pe_sum(L, P, f"S{it}")
                    if not final:
                        v = vrow(S, 1.0, "v1_t", P, np_=1)
                        nc.vector.tensor_add(st["w"][:], st["w"][:], v[:])
                        st["wb"] = bcast_row(st["w"], P, "i2")
                    else:
                        v = vrow(S, 1.0, "v2_t", P, np_=1)
                        fo = sch.tile([1, P, 10], F32, tag="fo_t", name="fo")
                        nc.scalar.activation(out=fo[:], in_=v[:], func=AF.Abs)
                        nc.sync.dma_start(
                            out=out_d[b0:b0 + P, :],
                            in_=fo[0:1, :, :],
                        )
                return cb

            if sched is None:
                s0_cb()
                iter_cb(1)()
                iter_cb(2)()
            else:
                inter, taps = sched
                inter.setdefault(taps[0], []).append(s0_cb)
                inter.setdefault(taps[1], []).append(iter_cb(1))
                inter.setdefault(taps[2], []).append(iter_cb(2))

        for _loop in range(loops):
            y_cur = yp.tile([128, 2, 20, 20, passes[0]], BF16, tag="y",
                            name="y_t")
            if _loop == 0:
                # head: xc0 already DMA'd above, then the big resident load
                load_wres()
                xc_cur = xc0_t
            else:
                xc_cur = xcols_load(0)
            for cb in conv0_ops(y_cur, xc_cur, passes[0]):
                cb()

            inter = {}
            for i, P in enumerate(passes):
                nxt = {}
                y_next = None
                if i + 1 < len(passes):
                    Pn = passes[i + 1]
                    y_next = yp.tile([128, 2, 20, 20, Pn], BF16, tag="y",
                                     name="y_t")
                    xc_box = []

                    def load_next(i=i):
                        xc_box.append(xcols_load(i + 1))

                    inter.setdefault(4, []).append(load_next)
                    cbs_box = []

                    def make_cbs(y_next=y_next, Pn=Pn):
                        cbs_box.extend(conv0_ops(y_next, xc_box[0], Pn))

                    inter.setdefault(24, []).append(make_cbs)
                    # spread conv0 chunks over taps 25..72
                    n_cb = 2 * ((400 * Pn + CHUNK - 1) // CHUNK)
                    for ci in range(n_cb):
                        def run_cb(ci=ci):
                            cbs_box[ci]()
                        inter.setdefault(25 + 2 * ci, []).append(run_cb)
                pr = prim_pass(y_cur, P, interleave=inter)
                u_t = stage2_pass(pr, P)
                if i + 1 < len(passes):
                    # defer the chain's PE-stream stages into the next
                    # pass's prim at taps where their inputs are ready
                    taps = (36, 54, 72) if passes[i + 1] == 14 else (56, 70, 79)
                    chain_sched(u_t, P, offs[i], sched=(nxt, taps))
                else:
                    chain_sched(u_t, P, offs[i], sched=None)
                inter = nxt
                y_cur = y_next

    nc.compile()
    return nc


# ---------------- host side ----------------

_CACHE = {}


def _prep(x, conv0_w, conv0_b, prim_w, prim_b, digit_w):
    B = x.shape[0]
    xw = np.lib.stride_tricks.sliding_window_view(x[:, 0], (9, 9), axis=(1, 2))
    # (B, 20, 20, 9, 9) -> (B, 9, 9, 20, 20) -> (B, 81, 400)
    xcols_std = np.ascontiguousarray(
        xw.transpose(0, 3, 4, 1, 2).reshape(B, 81, 400)
    )
    c0wT = np.ascontiguousarray(
        conv0_w.reshape(256, 81).T
    ).astype(ml_dtypes.bfloat16)
    c0b2 = np.ascontiguousarray(
        conv0_b.reshape(2, 128).T, dtype=np.float32
    )
    # prim weights resident layout [p, t, kt, r, q]: co = 2q+r, ci = kt*128+p
    pw = prim_w.reshape(128, 2, 2, 128, 81)  # (q, r, kt, p, t)
    wres = np.ascontiguousarray(
        pw.transpose(3, 4, 2, 1, 0)  # (p, t, kt, r, q)
    ).astype(ml_dtypes.bfloat16)
    pbr = np.ascontiguousarray(prim_b.reshape(128, 2), dtype=np.float32)
    dwr = np.ascontiguousarray(
        digit_w[:, :, 0, :].transpose(1, 0, 2).reshape(128, 9, 10, 8),
        dtype=np.float32,
    )
    return xcols_std, c0wT, c0b2, pbr, wres, dwr


def _xcols_interleave(xcols_core):
    """[Bc, 81, 400] -> [81, 400*Bc] with per-pass (pos, image) interleave."""
    Bc = xcols_core.shape[0]
    passes = PASSES if Bc == B_CORE else None
    if passes is None:
        out, rem = [], Bc
        while rem > 0:
            p = min(14, rem)
            out.append(p)
            rem -= p
        passes = tuple(out)
    blocks = []
    b0 = 0
    for P in passes:
        blk = xcols_core[b0:b0 + P]              # [P, 81, 400]
        blocks.append(blk.transpose(1, 2, 0).reshape(81, 400 * P))
        b0 += P
    return np.ascontiguousarray(
        np.concatenate(blocks, axis=1)
    ).astype(ml_dtypes.bfloat16)


def make_in_maps(x, conv0_w, conv0_b, prim_w, prim_b, digit_w):
    x = np.asarray(x, dtype=np.float32)
    conv0_w = np.asarray(conv0_w, dtype=np.float32)
    conv0_b = np.asarray(conv0_b, dtype=np.float32)
    prim_w = np.asarray(prim_w, dtype=np.float32)
    prim_b = np.asarray(prim_b, dtype=np.float32)
    digit_w = np.asarray(digit_w, dtype=np.float32)

    xcols_std, c0wT, c0b2, pbr, wres, dwr = _prep(
        x, conv0_w, conv0_b, prim_w, prim_b, digit_w
    )
    onesb = np.ones((128, 1), dtype=ml_dtypes.bfloat16)
    onesr = np.ones((1, 128), dtype=ml_dtypes.bfloat16)
    in_maps = []
    for c in range(N_CORES):
        sl = slice(c * B_CORE, (c + 1) * B_CORE)
        in_maps.append(
            {
                "xcols": _xcols_interleave(xcols_std[sl]),
                "c0wT": c0wT,
                "c0b2": c0b2,
                "pbr": pbr,
                "wres": wres,
                "dwr": dwr,
                "onesb": onesb,
                "onesr": onesr,
            }
        )
    return in_maps


def kernel(x, conv0_w, conv0_b, prim_w, prim_b, digit_w):
    from concourse.bass_utils import run_bass_kernel_spmd

    in_maps = make_in_maps(x, conv0_w, conv0_b, prim_w, prim_b, digit_w)

    if "nc" not in _CACHE:
        _CACHE["nc"] = build(B_CORE)
    nc = _CACHE["nc"]

    res = run_bass_kernel_spmd(nc, in_maps, core_ids=list(range(N_CORES)))
    out = np.concatenate([r["out"] for r in res.results], axis=0)
    return out.astype(np.float32)


if __name__ == "__main__":
    # quick smoke build
    nc = build()
    print("build ok")


# revision 29
# speedup vs baseline: 1.2121x; 1.0040x over previous
"""CapsuleNet forward kernel for 8 Trainium2 NeuronCores (pure data parallel).

Host side: im2col + weight-layout prep in numpy; batch 512 sharded 64/core.
Device side (per core), v3 design:
  - y stored P-inner [128, kt, 20, 20, P] so prim-conv rhs slices
    [6, 6, P] have a contiguous inner dim -> full-rate PE streaming
    (216ns/504-col matmul vs 302ns for the P-outer layout)
  - conv0 computed pass-wide from (pos, image)-interleaved im2col
    columns; PSUM evacuates with contiguous writes into the y layout
  - prim PSUM is (pos-major, image-minor) [128, 36, P]; evacuated by 4
    ACT copies straight into capsule-major u[q, j, b, k]
  - routing chain for overlapped passes runs on DVE + ACT + GPSIMD
    (partition sums via gpsimd.partition_all_reduce) so the PE streams
    uninterrupted; the LAST pass's chain instead uses PE ones-matmul
    partition+j reduction and gpsimd broadcast to cut the exposed tail
  - head: pass-0 xcols DMA'd before the 10.6MB resident weights
"""

import sys

if "/opt/trn_rl_repo" not in sys.path:
    sys.path.insert(0, "/opt/trn_rl_repo")

from contextlib import ExitStack

import ml_dtypes
import numpy as np

import concourse.bacc as bacc
import concourse.bass as bass
import concourse.bass_isa as bass_isa
import concourse.tile as tile
from concourse import mybir

F32 = mybir.dt.float32
BF16 = mybir.dt.bfloat16
AF = mybir.ActivationFunctionType
OP = mybir.AluOpType
RED = bass_isa.ReduceOp

N_CORES = 8
B_FULL = 512
B_CORE = B_FULL // N_CORES

PASSES = (14, 14, 14, 14, 8)
CHUNK = 512  # conv0 psum chunk columns


def build(B=B_CORE, passes=None, loops=1):
    """Build the Bass module for one core processing B images.

    loops>1 repeats the whole program (same output) for benchmarking."""
    if passes is None:
        passes = PASSES if B == B_CORE else None
    if passes is None:
        out, rem = [], B
        while rem > 0:
            p = min(14, rem)
            out.append(p)
            rem -= p
        passes = tuple(out)
    assert sum(passes) == B
    assert all(p <= 14 for p in passes)

    nc = bacc.Bacc("TRN2")

    # ---- DRAM I/O ----
    xcols_d = nc.dram_tensor("xcols", [81, 400 * B], BF16,
                             kind="ExternalInput")
    c0wT_d = nc.dram_tensor("c0wT", [81, 256], BF16, kind="ExternalInput")
    c0b_d = nc.dram_tensor("c0b2", [128, 2], F32, kind="ExternalInput")
    pbr_d = nc.dram_tensor("pbr", [128, 2], F32, kind="ExternalInput")
    # resident prim weights: [p(=ci%128), tap, kt(=ci//128), r(=co%2), q(=co//2)]
    wres_d = nc.dram_tensor("wres", [128, 81, 2, 2, 128], BF16,
                            kind="ExternalInput")
    # capsule weights: [m(=i//9), j(=i%9), o, k]
    dwr_d = nc.dram_tensor("dwr", [128, 9, 10, 8], F32, kind="ExternalInput")
    onesb_d = nc.dram_tensor("onesb", [128, 1], BF16, kind="ExternalInput")
    onesr_d = nc.dram_tensor("onesr", [1, 128], BF16, kind="ExternalInput")
    out_d = nc.dram_tensor("out", [B, 10], F32, kind="ExternalOutput")

    with ExitStack() as ctx:
        tc = ctx.enter_context(tile.TileContext(nc))

        consts = ctx.enter_context(tc.tile_pool(name="consts", bufs=1))
        xcp = ctx.enter_context(tc.tile_pool(name="xcp", bufs=2))
        yp = ctx.enter_context(tc.tile_pool(name="yp", bufs=2))
        up = ctx.enter_context(tc.tile_pool(name="up", bufs=2))
        xhp = ctx.enter_context(tc.tile_pool(name="xhp", bufs=2))
        tmpp = ctx.enter_context(tc.tile_pool(name="tmpp", bufs=1))
        smp = ctx.enter_context(tc.tile_pool(name="smp", bufs=1))
        sch = ctx.enter_context(tc.tile_pool(name="sch", bufs=1))
        pc0 = ctx.enter_context(tc.tile_pool(name="pc0", bufs=3, space="PSUM"))
        ppr = ctx.enter_context(tc.tile_pool(name="ppr", bufs=2, space="PSUM"))
        pch = ctx.enter_context(tc.tile_pool(name="pch", bufs=1, space="PSUM"))

        offs = []
        b0 = 0
        for P in passes:
            offs.append(b0)
            b0 += P

        # ---- head-critical DMAs first: conv0 needs c0wT + early xcols ----
        c0wT_t = consts.tile([81, 256], BF16)
        nc.sync.dma_start(out=c0wT_t, in_=c0wT_d[:, :])
        xc0_t = xcp.tile([81, 400 * passes[0]], BF16, tag="xc", name="xc_t")
        nc.sync.dma_start(out=xc0_t, in_=xcols_d[:, 0:400 * passes[0]])
        c0b_t = consts.tile([128, 2], F32)
        nc.sync.dma_start(out=c0b_t, in_=c0b_d[:, :])
        pb_t = consts.tile([128, 2], F32)
        nc.sync.dma_start(out=pb_t, in_=pbr_d[:, :])
        dwr_t = consts.tile([128, 9, 10, 8], F32)
        nc.sync.dma_start(out=dwr_t, in_=dwr_d[:, :, :, :])
        onesb_t = consts.tile([128, 1], BF16)
        nc.sync.dma_start(out=onesb_t, in_=onesb_d[:, :])
        onesr_t = consts.tile([1, 128], BF16)
        nc.sync.dma_start(out=onesr_t, in_=onesr_d[:, :])
        dwrb_t = consts.tile([128, 9, 10, 8], BF16)
        nc.vector.tensor_copy(out=dwrb_t[:], in_=dwr_t[:])

        def xcols_load(i):
            P = passes[i]
            xc_t = xcp.tile([81, 400 * P], BF16, tag="xc", name="xc_t")
            o = offs[i] * 400
            nc.sync.dma_start(out=xc_t, in_=xcols_d[:, o:o + 400 * P])
            return xc_t

        _CB = {"n": 0}

        def conv0_ops(y_t, xc_t, P, segs=None):
            """Pass-wide conv0: (pos,img)-interleaved columns; returns list
            of callbacks each doing one chunk matmul+relu-evac. segs: list
            of (tile, col_start, col_end) overriding the single xc_t."""
            total = 400 * P
            yf = y_t.rearrange("p t h w b -> p t (h w b)")
            if segs is None:
                segs = [(xc_t, 0, total)]

            def rhs_for(c0, cw):
                for t_, s_, e_ in segs:
                    if c0 >= s_ and c0 + cw <= e_:
                        return t_[:, c0 - s_:c0 - s_ + cw]
                raise AssertionError("chunk crosses segment boundary")

            cbs = []
            for mt in range(2):
                for c0 in range(0, total, CHUNK):
                    cw = min(CHUNK, total - c0)

                    def cb(mt=mt, c0=c0, cw=cw):
                        ps = pc0.tile([128, CHUNK], F32, tag="pc0",
                                      name="c0ps")
                        nc.tensor.matmul(
                            out=ps[:, 0:cw],
                            lhsT=c0wT_t[:, mt * 128:(mt + 1) * 128],
                            rhs=rhs_for(c0, cw),
                            start=True, stop=True,
                        )
                        nc.scalar.activation(
                            out=yf[:, mt, c0:c0 + cw], in_=ps[:, 0:cw],
                            func=AF.Relu, bias=c0b_t[:, mt:mt + 1],
                            scale=1.0,
                        )
                    cbs.append(cb)
            return cbs

        def conv0_ops_cmajor(y_t, xc_t, P):
            """conv0 chunk callbacks ordered chunk-major (mt pairs), so the
            low-position chunks needed by early prim taps evacuate first."""
            cbs = conv0_ops(y_t, xc_t, P)
            n = len(cbs) // 2
            out = []
            for c in range(n):
                out.append(cbs[c])
                out.append(cbs[n + c])
            return out

        wres_ts = []

        def load_wres():
            # per-chunk weight tiles so early prim taps don't wait on the
            # full 10.6MB resident load
            for t0 in range(0, 81, 9):
                wt = consts.tile([128, 9, 2, 2, 128], BF16,
                                 name=f"wres{t0}")
                nc.sync.dma_start(out=wt, in_=wres_d[:, t0:t0 + 9])
                wres_ts.append(wt)

        def prim_pass(y_t, P, interleave=None):
            """prim caps conv: accumulate 81 taps x 2kt into 2 r-PSUM tiles
            shaped [128, 6, 6, P] (pos-major, image-minor)."""
            pr = [ppr.tile([128, 6, 6, P], F32, tag=f"ppr{r}",
                           name=f"ppr_{r}") for r in range(2)]
            for t in range(81):
                if interleave and t in interleave:
                    for cb in interleave[t]:
                        cb()
                kh, kw = t // 9, t % 9
                wt = wres_ts[t // 9]
                for kt in range(2):
                    for r in range(2):
                        rhs = y_t[:, kt, kh:kh + 12:2, kw:kw + 12:2, :]
                        nc.tensor.matmul(
                            out=pr[r].rearrange("p h w b -> p (h w b)"),
                            lhsT=wt[:, t % 9, kt, r, :],
                            rhs=rhs,
                            start=(t == 0 and kt == 0),
                            stop=(t == 80 and kt == 1),
                        )
            return pr

        def stage2_pass(pr, P):
            """evacuate prim PSUM (+bias) into capsule-major u[q, j, b, k].
            PSUM columns are (pos, b); 36r+pos = 8j+k."""
            u_t = up.tile([128, 9, P, 8], F32, tag="u", name="u_t")
            V0 = pr[0].rearrange("p h w b -> p (h w) b")
            V1 = pr[1].rearrange("p h w b -> p (h w) b")
            nc.scalar.activation(
                out=u_t[:, 0:4, :, :],
                in_=V0[:, 0:32, :].rearrange("p (j k) b -> p j k b", k=8)
                .transpose([0, 1, 3, 2]),
                func=AF.Identity, bias=pb_t[:, 0:1], scale=1.0,
            )
            nc.scalar.activation(
                out=u_t[:, 4, :, 0:4],
                in_=V0[:, 32:36, :].transpose([0, 2, 1]),
                func=AF.Identity, bias=pb_t[:, 0:1], scale=1.0,
            )
            nc.scalar.activation(
                out=u_t[:, 4, :, 4:8],
                in_=V1[:, 0:4, :].transpose([0, 2, 1]),
                func=AF.Identity, bias=pb_t[:, 1:2], scale=1.0,
            )
            nc.scalar.activation(
                out=u_t[:, 5:9, :, :],
                in_=V1[:, 4:36, :].rearrange("p (j k) b -> p j k b", k=8)
                .transpose([0, 1, 3, 2]),
                func=AF.Identity, bias=pb_t[:, 1:2], scale=1.0,
            )
            return u_t

        def vrow(S_t, alpha, tag, P, np_=128):
            """squash scalar per (b,o): v = s^3/((1+s^2)(|s|+eps)), s=alpha*S.
            Works on [np_, P, 10] tiles (np_=128 replicated, or 1)."""
            sh = [np_, P, 10]
            pool = smp if np_ == 128 else sch
            if alpha != 1.0:
                ts_ = pool.tile(sh, F32, tag="vr_ts", name="vr_ts")
                nc.scalar.activation(out=ts_[:], in_=S_t[:], func=AF.Copy,
                                     scale=alpha)
            else:
                ts_ = S_t
            s2 = pool.tile(sh, F32, tag="vr_s2", name="vr_s2")
            nc.scalar.activation(out=s2[:], in_=S_t[:], func=AF.Square,
                                 scale=alpha)
            ab = pool.tile(sh, F32, tag="vr_ab", name="vr_ab")
            nc.scalar.activation(out=ab[:], in_=S_t[:], func=AF.Abs,
                                 scale=alpha)
            nc.vector.scalar_tensor_tensor(
                out=ab[:], in0=s2[:], scalar=1.0, in1=ab[:],
                op0=OP.add, op1=OP.mult,
            )
            nc.vector.tensor_scalar_add(ab[:], ab[:], 1e-8)
            nc.vector.reciprocal_approx_fast(out=ab[:], in_=ab[:])
            nc.vector.tensor_mul(s2[:], ts_[:], s2[:])
            v = pool.tile(sh, F32, tag=tag, name=tag)
            nc.vector.tensor_mul(v[:], s2[:], ab[:])
            return v

        def squash_u(u_t, P):
            """in-place squash of u over k plus bf16 copy + x_hat."""
            usq = tmpp.tile([128, 9, P, 8], F32, tag="usq", name="usq")
            nc.scalar.activation(out=usq[:], in_=u_t[:], func=AF.Square)
            n2 = smp.tile([128, 9, P], F32, tag="n2", name="n2")
            nc.vector.tensor_reduce(out=n2[:], in_=usq[:],
                                    axis=mybir.AxisListType.X, op=OP.add)
            nrm = smp.tile([128, 9, P], F32, tag="nrm", name="nrm")
            nc.scalar.activation(out=nrm[:], in_=n2[:], func=AF.Sqrt)
            nc.vector.tensor_scalar_add(n2[:], n2[:], 1.0)
            nc.vector.reciprocal_approx_fast(out=n2[:], in_=n2[:])
            nc.vector.tensor_mul(nrm[:], nrm[:], n2[:])
            ub = tmpp.tile([128, 9, P, 8], BF16, tag="ub", name="ub")
            with nc.allow_low_precision(reason="bf16 squashed u"):
                nc.vector.tensor_mul(
                    ub[:], u_t[:],
                    nrm.unsqueeze(3).broadcast_to([128, 9, P, 8]),
                )

            X_t = xhp.tile([128, 9, P, 10], BF16, tag="X", name="X_t")
            with nc.allow_low_precision(reason="bf16 x_hat (8-term dots)"):
                for o in range(10):
                    xt = tmpp.tile([128, 9, P, 8], BF16, tag="xh",
                                   name="xh_tmp")
                    nc.vector.tensor_mul(
                        xt[:], ub[:],
                        dwrb_t[:, :, o, :].unsqueeze(2)
                        .broadcast_to([128, 9, P, 8]),
                    )
                    nc.vector.tensor_reduce(out=X_t[:, :, :, o], in_=xt[:],
                                            axis=mybir.AxisListType.X,
                                            op=OP.add)
            return X_t

        def pe_sum(src_t, P, name, scale01=False):
            """S[1, P*10] = sum over partitions and j of src[128, 9, P, 10]
            via 9 accumulating ones-matmuls; evac to a [1, P, 10] f32 tile.
            scale01: use the 0.1-valued lhsT column (uniform softmax S0)."""
            col = 1 if scale01 else 0
            ps = pch.tile([1, P * 10], F32, tag="pchS", name=name + "p")
            for j in range(9):
                nc.tensor.matmul(
                    out=ps[:, :],
                    lhsT=onesb_t[:, col:col + 1],
                    rhs=src_t[:, j].rearrange("p b o -> p (b o)"),
                    start=(j == 0), stop=(j == 8),
                )
            Sr = sch.tile([1, P, 10], F32, tag="Sr_" + name, name=name)
            nc.scalar.activation(out=Sr.rearrange("p b o -> p (b o)"),
                                 in_=ps[:, :], func=AF.Identity, scale=1.0)
            return Sr

        def bcast_row(vrow_t, P, name):
            """broadcast [1, P, 10] f32 -> [128, P, 10] bf16 via an
            outer-product ones-matmul (PE) + ACT evac."""
            vb = sch.tile([1, P, 10], BF16, tag="vb_" + name, name="vb")
            nc.vector.tensor_copy(out=vb[:], in_=vrow_t[:])
            ps = pch.tile([128, P * 10], F32, tag="pchS", name="bc_" + name)
            nc.tensor.matmul(
                out=ps[:, :],
                lhsT=onesr_t[:, :],
                rhs=vb.rearrange("p b o -> p (b o)"),
                start=True, stop=True,
            )
            wb = smp.tile([128, P, 10], BF16, tag="wbt", name="wb_" + name)
            nc.scalar.activation(out=wb.rearrange("p b o -> p (b o)"),
                                 in_=ps[:, :], func=AF.Identity, scale=1.0)
            return wb

        def chain_stages(X_t, P, b0):
            """Make the S0/iter1/iter2 stage callbacks for one X block."""
            st = {}

            def s0_cb():
                S0 = pe_sum(X_t, P, "S0")
                st["w"] = vrow(S0, 0.1, "w_acc_t", P, np_=1)
                st["wb"] = bcast_row(st["w"], P, "i1")

            def iter_cb(it):
                final = it == 2

                def cb():
                    wb = st["wb"]
                    L = tmpp.tile([128, 9, P, 10], BF16, tag="L", name="Lt")
                    with nc.allow_low_precision(reason="bf16 routing logits"):
                        nc.vector.tensor_mul(
                            L[:], X_t[:],
                            wb.unsqueeze(1).broadcast_to([128, 9, P, 10]),
                        )
                    nc.scalar.activation(out=L[:], in_=L[:], func=AF.Exp)
                    Z = smp.tile([128, 9, P], F32, tag="Z", name="Zt")
                    nc.vector.tensor_reduce(out=Z[:], in_=L[:],
                                            axis=mybir.AxisListType.X,
                                            op=OP.add)
                    rZf = smp.tile([128, 9, P], F32, tag="rZf", name="rZt")
                    nc.vector.reciprocal_approx_fast(out=rZf[:], in_=Z[:])
                    rZ = smp.tile([128, 9, P], BF16, tag="rZ", name="rZb")
                    with nc.allow_low_precision(reason="bf16 softmax denom"):
                        nc.vector.tensor_copy(out=rZ[:], in_=rZf[:])
                        nc.vector.tensor_mul(L[:], L[:], X_t[:])
                        nc.vector.tensor_mul(
                            L[:], L[:],
                            rZ.unsqueeze(3).broadcast_to([128, 9, P, 10]),
                        )
                    S = pe_sum(L, P, f"S{it}")
                    if not final:
                        v = vrow(S, 1.0, "v1_t", P, np_=1)
                        nc.vector.tensor_add(st["w"][:], st["w"][:], v[:])
                        st["wb"] = bcast_row(st["w"], P, "i2")
                    else:
                        v = vrow(S, 1.0, "v2_t", P, np_=1)
                        fo = sch.tile([1, P, 10], F32, tag="fo_t", name="fo")
                        nc.scalar.activation(out=fo[:], in_=v[:], func=AF.Abs)
                        nc.sync.dma_start(
                            out=out_d[b0:b0 + P, :],
                            in_=fo[0:1, :, :],
                        )
                return cb

            return s0_cb, iter_cb(1), iter_cb(2)

        def chain_sched(u_t, P, b0, sched=None):
            """squash -> x_hat -> 3 routing iterations -> |v| -> out DMA.
            Partition(+j) sums run on PE via ones-matmuls; vrow on a single
            partition. When `sched` (an interleave dict for the NEXT pass's
            prim, plus tap positions) is given, the S/iter stages are
            deferred so the PE-stream ops land at taps where their DVE
            inputs are surely ready. The inline (last-pass) path splits P
            into two half-blocks pipelined across engines: block B's
            DVE-heavy squash/x_hat overlaps block A's iteration stages."""
            if sched is not None:
                X_t = squash_u(u_t, P)
                s0, i1, i2 = chain_stages(X_t, P, b0)
                inter, taps = sched
                inter.setdefault(taps[0], []).append(s0)
                inter.setdefault(taps[1], []).append(i1)
                inter.setdefault(taps[2], []).append(i2)
                return
            X_t = squash_u(u_t, P)
            s0, i1, i2 = chain_stages(X_t, P, b0)
            s0()
            i1()
            i2()

        for _loop in range(loops):
            y_cur = yp.tile([128, 2, 20, 20, passes[0]], BF16, tag="y",
                            name="y_t")
            if _loop == 0:
                # head: xc0 already DMA'd above, then the big resident load
                load_wres()
                xc_cur = xc0_t
            else:
                xc_cur = xcols_load(0)
            # lead-in: emit only the chunk pairs prim tap (0,0) needs
            # (positions up to (10,10) -> flat 210 -> chunk ceil(211*P/512));
            # the rest interleave into the first prim taps.
            cbs0 = conv0_ops_cmajor(y_cur, xc_cur, passes[0])
            need = 2 * ((211 * passes[0] + CHUNK - 1) // CHUNK)
            for cb in cbs0[:need]:
                cb()

            inter = {}
            for ci, cb in enumerate(cbs0[need:]):
                inter.setdefault(1 + ci, []).append(cb)
            for i, P in enumerate(passes):
                nxt = {}
                y_next = None
                if i + 1 < len(passes):
                    Pn = passes[i + 1]
                    y_next = yp.tile([128, 2, 20, 20, Pn], BF16, tag="y",
                                     name="y_t")
                    xc_box = []

                    def load_next(i=i):
                        xc_box.append(xcols_load(i + 1))

                    inter.setdefault(4, []).append(load_next)
                    cbs_box = []

                    def make_cbs(y_next=y_next, Pn=Pn):
                        cbs_box.extend(conv0_ops(y_next, xc_box[0], Pn))

                    inter.setdefault(24, []).append(make_cbs)
                    # spread conv0 chunks over taps 25..72
                    n_cb = 2 * ((400 * Pn + CHUNK - 1) // CHUNK)
                    for ci in range(n_cb):
                        def run_cb(ci=ci):
                            cbs_box[ci]()
                        inter.setdefault(25 + 2 * ci, []).append(run_cb)
                pr = prim_pass(y_cur, P, interleave=inter)
                u_t = stage2_pass(pr, P)
                if i + 1 < len(passes):
                    # defer the chain's PE-stream stages into the next
                    # pass's prim at taps where their inputs are ready
                    taps = (36, 54, 72) if passes[i + 1] == 14 else (56, 70, 79)
                    chain_sched(u_t, P, offs[i], sched=(nxt, taps))
                else:
                    chain_sched(u_t, P, offs[i], sched=None)
                inter = nxt
                y_cur = y_next

    nc.compile()
    return nc


# ---------------- host side ----------------

_CACHE = {}


def _prep(x, conv0_w, conv0_b, prim_w, prim_b, digit_w):
    B = x.shape[0]
    xw = np.lib.stride_tricks.sliding_window_view(x[:, 0], (9, 9), axis=(1, 2))
    # (B, 20, 20, 9, 9) -> (B, 9, 9, 20, 20) -> (B, 81, 400)
    xcols_std = np.ascontiguousarray(
        xw.transpose(0, 3, 4, 1, 2).reshape(B, 81, 400)
    )
    c0wT = np.ascontiguousarray(
        conv0_w.reshape(256, 81).T
    ).astype(ml_dtypes.bfloat16)
    c0b2 = np.ascontiguousarray(
        conv0_b.reshape(2, 128).T, dtype=np.float32
    )
    # prim weights resident layout [p, t, kt, r, q]: co = 2q+r, ci = kt*128+p
    pw = prim_w.reshape(128, 2, 2, 128, 81)  # (q, r, kt, p, t)
    wres = np.ascontiguousarray(
        pw.transpose(3, 4, 2, 1, 0)  # (p, t, kt, r, q)
    ).astype(ml_dtypes.bfloat16)
    pbr = np.ascontiguousarray(prim_b.reshape(128, 2), dtype=np.float32)
    dwr = np.ascontiguousarray(
        digit_w[:, :, 0, :].transpose(1, 0, 2).reshape(128, 9, 10, 8),
        dtype=np.float32,
    )
    return xcols_std, c0wT, c0b2, pbr, wres, dwr


def _xcols_interleave(xcols_core):
    """[Bc, 81, 400] -> [81, 400*Bc] with per-pass (pos, image) interleave."""
    Bc = xcols_core.shape[0]
    passes = PASSES if Bc == B_CORE else None
    if passes is None:
        out, rem = [], Bc
        while rem > 0:
            p = min(14, rem)
            out.append(p)
            rem -= p
        passes = tuple(out)
    blocks = []
    b0 = 0
    for P in passes:
        blk = xcols_core[b0:b0 + P]              # [P, 81, 400]
        blocks.append(blk.transpose(1, 2, 0).reshape(81, 400 * P))
        b0 += P
    return np.ascontiguousarray(
        np.concatenate(blocks, axis=1)
    ).astype(ml_dtypes.bfloat16)


def make_in_maps(x, conv0_w, conv0_b, prim_w, prim_b, digit_w):
    x = np.asarray(x, dtype=np.float32)
    conv0_w = np.asarray(conv0_w, dtype=np.float32)
    conv0_b = np.asarray(conv0_b, dtype=np.float32)
    prim_w = np.asarray(prim_w, dtype=np.float32)
    prim_b = np.asarray(prim_b, dtype=np.float32)
    digit_w = np.asarray(digit_w, dtype=np.float32)

    xcols_std, c0wT, c0b2, pbr, wres, dwr = _prep(
        x, conv0_w, conv0_b, prim_w, prim_b, digit_w
    )
    onesb = np.ones((128, 1), dtype=ml_dtypes.bfloat16)
    onesr = np.ones((1, 128), dtype=ml_dtypes.bfloat16)
    in_maps = []
    for c in range(N_CORES):
        sl = slice(c * B_CORE, (c + 1) * B_CORE)
        in_maps.append(
            {
                "xcols": _xcols_interleave(xcols_std[sl]),
                "c0wT": c0wT,
                "c0b2": c0b2,
                "pbr": pbr,
                "wres": wres,
                "dwr": dwr,
                "onesb": onesb,
                "onesr": onesr,
            }
        )
    return in_maps


def kernel(x, conv0_w, conv0_b, prim_w, prim_b, digit_w):
    from concourse.bass_utils import run_bass_kernel_spmd

    in_maps = make_in_maps(x, conv0_w, conv0_b, prim_w, prim_b, digit_w)

    if "nc" not in _CACHE:
        _CACHE["nc"] = build(B_CORE)
    nc = _CACHE["nc"]

    res = run_bass_kernel_spmd(nc, in_maps, core_ids=list(range(N_CORES)))
    out = np.concatenate([r["out"] for r in res.results], axis=0)
    return out.astype(np.float32)


if __name__ == "__main__":
    # quick smoke build
    nc = build()
    print("build ok")


# revision 30
# speedup vs baseline: 1.2139x; 1.0015x over previous
"""CapsuleNet forward kernel for 8 Trainium2 NeuronCores (pure data parallel).

Host side: im2col + weight-layout prep in numpy; batch 512 sharded 64/core.
Device side (per core), v4 design:
  - y stored P-inner [128, kt, 20, 20, P] so prim-conv rhs slices
    [6, 6, P] have a contiguous inner dim -> full-rate PE streaming
    (216ns/504-col matmul vs 302ns for the P-outer layout). NOTE: prim
    matmul speed is sensitive to the SBUF placement of y/wres — pool
    and tag changes in consts/xcp/yp shift addresses and have caused
    reproducible 216->259ns regressions; re-profile after any change.
  - conv0 computed pass-wide from (pos, image)-interleaved im2col
    columns; PSUM evacuates with contiguous writes into the y layout.
    Pass-0 lead-in emits only the chunks prim tap (0,0) needs; the
    rest interleave into the first prim taps.
  - prim PSUM is (pos-major, image-minor) [128, 36, P]; evacuated by 4
    ACT copies straight into capsule-major u[q, j, b, k]
  - routing chain: partition(+j) sums via PE ones-matmuls, squash rows
    (vrow) on a single partition, broadcast back via outer-product
    ones-matmul. For overlapped passes the S/iter stages defer into
    the NEXT pass's prim tap stream at taps where their DVE inputs are
    ready, so the PE never stalls mid-stream; only the last pass's
    chain is exposed (~45us, DVE-serial).
  - head: c0wT + pass-0 xcols DMA'd before the 10.6MB resident weights
"""

import sys

if "/opt/trn_rl_repo" not in sys.path:
    sys.path.insert(0, "/opt/trn_rl_repo")

from contextlib import ExitStack

import ml_dtypes
import numpy as np

import concourse.bacc as bacc
import concourse.bass as bass
import concourse.bass_isa as bass_isa
import concourse.tile as tile
from concourse import mybir

F32 = mybir.dt.float32
BF16 = mybir.dt.bfloat16
AF = mybir.ActivationFunctionType
OP = mybir.AluOpType
RED = bass_isa.ReduceOp

N_CORES = 8
B_FULL = 512
B_CORE = B_FULL // N_CORES

PASSES = (14, 14, 14, 14, 8)
CHUNK = 512  # conv0 psum chunk columns


def build(B=B_CORE, passes=None, loops=1):
    """Build the Bass module for one core processing B images.

    loops>1 repeats the whole program (same output) for benchmarking."""
    if passes is None:
        passes = PASSES if B == B_CORE else None
    if passes is None:
        out, rem = [], B
        while rem > 0:
            p = min(14, rem)
            out.append(p)
            rem -= p
        passes = tuple(out)
    assert sum(passes) == B
    assert all(p <= 14 for p in passes)

    nc = bacc.Bacc("TRN2")

    # ---- DRAM I/O ----
    xcols_d = nc.dram_tensor("xcols", [81, 400 * B], BF16,
                             kind="ExternalInput")
    c0wT_d = nc.dram_tensor("c0wT", [81, 256], BF16, kind="ExternalInput")
    c0b_d = nc.dram_tensor("c0b2", [128, 2], F32, kind="ExternalInput")
    pbr_d = nc.dram_tensor("pbr", [128, 2], F32, kind="ExternalInput")
    # resident prim weights: [p(=ci%128), tap, kt(=ci//128), r(=co%2), q(=co//2)]
    wres_d = nc.dram_tensor("wres", [128, 81, 2, 2, 128], BF16,
                            kind="ExternalInput")
    # capsule weights: [m(=i//9), j(=i%9), o, k]
    dwr_d = nc.dram_tensor("dwr", [128, 9, 10, 8], F32, kind="ExternalInput")
    onesb_d = nc.dram_tensor("onesb", [128, 1], BF16, kind="ExternalInput")
    onesr_d = nc.dram_tensor("onesr", [1, 128], BF16, kind="ExternalInput")
    out_d = nc.dram_tensor("out", [B, 10], F32, kind="ExternalOutput")

    with ExitStack() as ctx:
        tc = ctx.enter_context(tile.TileContext(nc))

        consts = ctx.enter_context(tc.tile_pool(name="consts", bufs=1))
        xcp = ctx.enter_context(tc.tile_pool(name="xcp", bufs=2))
        yp = ctx.enter_context(tc.tile_pool(name="yp", bufs=2))
        up = ctx.enter_context(tc.tile_pool(name="up", bufs=2))
        xhp = ctx.enter_context(tc.tile_pool(name="xhp", bufs=2))
        tmpp = ctx.enter_context(tc.tile_pool(name="tmpp", bufs=1))
        smp = ctx.enter_context(tc.tile_pool(name="smp", bufs=1))
        sch = ctx.enter_context(tc.tile_pool(name="sch", bufs=1))
        pc0 = ctx.enter_context(tc.tile_pool(name="pc0", bufs=3, space="PSUM"))
        ppr = ctx.enter_context(tc.tile_pool(name="ppr", bufs=2, space="PSUM"))
        pch = ctx.enter_context(tc.tile_pool(name="pch", bufs=1, space="PSUM"))

        offs = []
        b0 = 0
        for P in passes:
            offs.append(b0)
            b0 += P

        # ---- head-critical DMAs first: conv0 needs c0wT + early xcols ----
        c0wT_t = consts.tile([81, 256], BF16)
        nc.sync.dma_start(out=c0wT_t, in_=c0wT_d[:, :])
        xc0_t = xcp.tile([81, 400 * passes[0]], BF16, tag="xc", name="xc_t")
        nc.sync.dma_start(out=xc0_t, in_=xcols_d[:, 0:400 * passes[0]])
        c0b_t = consts.tile([128, 2], F32)
        nc.sync.dma_start(out=c0b_t, in_=c0b_d[:, :])
        pb_t = consts.tile([128, 2], F32)
        nc.sync.dma_start(out=pb_t, in_=pbr_d[:, :])
        dwr_t = consts.tile([128, 9, 10, 8], F32)
        nc.sync.dma_start(out=dwr_t, in_=dwr_d[:, :, :, :])
        onesb_t = consts.tile([128, 1], BF16)
        nc.sync.dma_start(out=onesb_t, in_=onesb_d[:, :])
        onesr_t = consts.tile([1, 128], BF16)
        nc.sync.dma_start(out=onesr_t, in_=onesr_d[:, :])
        dwrb_t = consts.tile([128, 9, 10, 8], BF16)
        nc.vector.tensor_copy(out=dwrb_t[:], in_=dwr_t[:])

        def xcols_load(i):
            P = passes[i]
            xc_t = xcp.tile([81, 400 * P], BF16, tag="xc", name="xc_t")
            o = offs[i] * 400
            nc.sync.dma_start(out=xc_t, in_=xcols_d[:, o:o + 400 * P])
            return xc_t

        _CB = {"n": 0}

        def conv0_ops(y_t, xc_t, P, segs=None):
            """Pass-wide conv0: (pos,img)-interleaved columns; returns list
            of callbacks each doing one chunk matmul+relu-evac. segs: list
            of (tile, col_start, col_end) overriding the single xc_t."""
            total = 400 * P
            yf = y_t.rearrange("p t h w b -> p t (h w b)")
            if segs is None:
                segs = [(xc_t, 0, total)]

            def rhs_for(c0, cw):
                for t_, s_, e_ in segs:
                    if c0 >= s_ and c0 + cw <= e_:
                        return t_[:, c0 - s_:c0 - s_ + cw]
                raise AssertionError("chunk crosses segment boundary")

            cbs = []
            for mt in range(2):
                for c0 in range(0, total, CHUNK):
                    cw = min(CHUNK, total - c0)

                    def cb(mt=mt, c0=c0, cw=cw):
                        ps = pc0.tile([128, CHUNK], F32, tag="pc0",
                                      name="c0ps")
                        nc.tensor.matmul(
                            out=ps[:, 0:cw],
                            lhsT=c0wT_t[:, mt * 128:(mt + 1) * 128],
                            rhs=rhs_for(c0, cw),
                            start=True, stop=True,
                        )
                        nc.scalar.activation(
                            out=yf[:, mt, c0:c0 + cw], in_=ps[:, 0:cw],
                            func=AF.Relu, bias=c0b_t[:, mt:mt + 1],
                            scale=1.0,
                        )
                    cbs.append(cb)
            return cbs

        def conv0_ops_cmajor(y_t, xc_t, P):
            """conv0 chunk callbacks ordered chunk-major (mt pairs), so the
            low-position chunks needed by early prim taps evacuate first."""
            cbs = conv0_ops(y_t, xc_t, P)
            n = len(cbs) // 2
            out = []
            for c in range(n):
                out.append(cbs[c])
                out.append(cbs[n + c])
            return out

        wres_ts = []

        def load_wres():
            # per-chunk weight tiles so early prim taps don't wait on the
            # full 10.6MB resident load
            for t0 in range(0, 81, 9):
                wt = consts.tile([128, 9, 2, 2, 128], BF16,
                                 name=f"wres{t0}")
                nc.sync.dma_start(out=wt, in_=wres_d[:, t0:t0 + 9])
                wres_ts.append(wt)

        def prim_pass(y_t, P, interleave=None):
            """prim caps conv: accumulate 81 taps x 2kt into 2 r-PSUM tiles
            shaped [128, 6, 6, P] (pos-major, image-minor)."""
            pr = [ppr.tile([128, 6, 6, P], F32, tag=f"ppr{r}",
                           name=f"ppr_{r}") for r in range(2)]
            for t in range(81):
                if interleave and t in interleave:
                    for cb in interleave[t]:
                        cb()
                kh, kw = t // 9, t % 9
                wt = wres_ts[t // 9]
                for kt in range(2):
                    for r in range(2):
                        rhs = y_t[:, kt, kh:kh + 12:2, kw:kw + 12:2, :]
                        nc.tensor.matmul(
                            out=pr[r].rearrange("p h w b -> p (h w b)"),
                            lhsT=wt[:, t % 9, kt, r, :],
                            rhs=rhs,
                            start=(t == 0 and kt == 0),
                            stop=(t == 80 and kt == 1),
                        )
            return pr

        def stage2_pass(pr, P):
            """evacuate prim PSUM (+bias) into capsule-major u[q, j, b, k].
            PSUM columns are (pos, b); 36r+pos = 8j+k."""
            u_t = up.tile([128, 9, P, 8], F32, tag="u", name="u_t")
            V0 = pr[0].rearrange("p h w b -> p (h w) b")
            V1 = pr[1].rearrange("p h w b -> p (h w) b")
            nc.scalar.activation(
                out=u_t[:, 0:4, :, :],
                in_=V0[:, 0:32, :].rearrange("p (j k) b -> p j k b", k=8)
                .transpose([0, 1, 3, 2]),
                func=AF.Identity, bias=pb_t[:, 0:1], scale=1.0,
            )
            nc.scalar.activation(
                out=u_t[:, 4, :, 0:4],
                in_=V0[:, 32:36, :].transpose([0, 2, 1]),
                func=AF.Identity, bias=pb_t[:, 0:1], scale=1.0,
            )
            nc.scalar.activation(
                out=u_t[:, 4, :, 4:8],
                in_=V1[:, 0:4, :].transpose([0, 2, 1]),
                func=AF.Identity, bias=pb_t[:, 1:2], scale=1.0,
            )
            nc.scalar.activation(
                out=u_t[:, 5:9, :, :],
                in_=V1[:, 4:36, :].rearrange("p (j k) b -> p j k b", k=8)
                .transpose([0, 1, 3, 2]),
                func=AF.Identity, bias=pb_t[:, 1:2], scale=1.0,
            )
            return u_t

        def vrow(S_t, alpha, tag, P, np_=128):
            """squash scalar per (b,o): v = s^3/((1+s^2)(|s|+eps)), s=alpha*S.
            Works on [np_, P, 10] tiles (np_=128 replicated, or 1)."""
            sh = [np_, P, 10]
            pool = smp if np_ == 128 else sch
            if alpha != 1.0:
                ts_ = pool.tile(sh, F32, tag="vr_ts", name="vr_ts")
                nc.scalar.activation(out=ts_[:], in_=S_t[:], func=AF.Copy,
                                     scale=alpha)
            else:
                ts_ = S_t
            s2 = pool.tile(sh, F32, tag="vr_s2", name="vr_s2")
            nc.scalar.activation(out=s2[:], in_=S_t[:], func=AF.Square,
                                 scale=alpha)
            ab = pool.tile(sh, F32, tag="vr_ab", name="vr_ab")
            nc.scalar.activation(out=ab[:], in_=S_t[:], func=AF.Abs,
                                 scale=alpha)
            nc.vector.scalar_tensor_tensor(
                out=ab[:], in0=s2[:], scalar=1.0, in1=ab[:],
                op0=OP.add, op1=OP.mult,
            )
            nc.vector.tensor_scalar_add(ab[:], ab[:], 1e-8)
            nc.vector.reciprocal_approx_fast(out=ab[:], in_=ab[:])
            nc.vector.tensor_mul(s2[:], ts_[:], s2[:])
            v = pool.tile(sh, F32, tag=tag, name=tag)
            nc.vector.tensor_mul(v[:], s2[:], ab[:])
            return v

        def squash_u(u_t, P):
            """in-place squash of u over k plus bf16 copy + x_hat."""
            usq = tmpp.tile([128, 9, P, 8], F32, tag="usq", name="usq")
            nc.scalar.activation(out=usq[:], in_=u_t[:], func=AF.Square)
            n2 = smp.tile([128, 9, P], F32, tag="n2", name="n2")
            nc.vector.tensor_reduce(out=n2[:], in_=usq[:],
                                    axis=mybir.AxisListType.X, op=OP.add)
            nrm = smp.tile([128, 9, P], F32, tag="nrm", name="nrm")
            nc.scalar.activation(out=nrm[:], in_=n2[:], func=AF.Sqrt)
            nc.vector.tensor_scalar_add(n2[:], n2[:], 1.0)
            nc.vector.reciprocal_approx_fast(out=n2[:], in_=n2[:])
            nc.vector.tensor_mul(nrm[:], nrm[:], n2[:])
            ub = tmpp.tile([128, 9, P, 8], BF16, tag="ub", name="ub")
            with nc.allow_low_precision(reason="bf16 squashed u"):
                nc.vector.tensor_mul(
                    ub[:], u_t[:],
                    nrm.unsqueeze(3).broadcast_to([128, 9, P, 8]),
                )

            X_t = xhp.tile([128, 9, P, 10], BF16, tag="X", name="X_t")
            with nc.allow_low_precision(reason="bf16 x_hat (8-term dots)"):
                for o in range(10):
                    xt = tmpp.tile([128, 9, P, 8], BF16, tag="xh",
                                   name="xh_tmp")
                    nc.vector.tensor_mul(
                        xt[:], ub[:],
                        dwrb_t[:, :, o, :].unsqueeze(2)
                        .broadcast_to([128, 9, P, 8]),
                    )
                    nc.vector.tensor_reduce(out=X_t[:, :, :, o], in_=xt[:],
                                            axis=mybir.AxisListType.X,
                                            op=OP.add)
            return X_t

        def pe_sum(src_t, P, name, scale01=False):
            """S[1, P*10] = sum over partitions and j of src[128, 9, P, 10]
            via 9 accumulating ones-matmuls; evac to a [1, P, 10] f32 tile.
            scale01: use the 0.1-valued lhsT column (uniform softmax S0)."""
            col = 1 if scale01 else 0
            ps = pch.tile([1, P * 10], F32, tag="pchS", name=name + "p")
            for j in range(9):
                nc.tensor.matmul(
                    out=ps[:, :],
                    lhsT=onesb_t[:, col:col + 1],
                    rhs=src_t[:, j].rearrange("p b o -> p (b o)"),
                    start=(j == 0), stop=(j == 8),
                )
            Sr = sch.tile([1, P, 10], F32, tag="Sr_" + name, name=name)
            nc.scalar.activation(out=Sr.rearrange("p b o -> p (b o)"),
                                 in_=ps[:, :], func=AF.Identity, scale=1.0)
            return Sr

        def bcast_row(vrow_t, P, name):
            """broadcast [1, P, 10] f32 -> [128, P, 10] bf16 via an
            outer-product ones-matmul (PE) + ACT evac."""
            vb = sch.tile([1, P, 10], BF16, tag="vb_" + name, name="vb")
            nc.vector.tensor_copy(out=vb[:], in_=vrow_t[:])
            ps = pch.tile([128, P * 10], F32, tag="pchS", name="bc_" + name)
            nc.tensor.matmul(
                out=ps[:, :],
                lhsT=onesr_t[:, :],
                rhs=vb.rearrange("p b o -> p (b o)"),
                start=True, stop=True,
            )
            wb = smp.tile([128, P, 10], BF16, tag="wbt", name="wb_" + name)
            nc.scalar.activation(out=wb.rearrange("p b o -> p (b o)"),
                                 in_=ps[:, :], func=AF.Identity, scale=1.0)
            return wb

        def chain_stages(X_t, P, b0):
            """Make the S0/iter1/iter2 stage callbacks for one X block."""
            st = {}

            def s0_cb():
                S0 = pe_sum(X_t, P, "S0")
                st["w"] = vrow(S0, 0.1, "w_acc_t", P, np_=1)
                st["wb"] = bcast_row(st["w"], P, "i1")

            def iter_cb(it):
                final = it == 2

                def cb():
                    wb = st["wb"]
                    L = tmpp.tile([128, 9, P, 10], BF16, tag="L", name="Lt")
                    with nc.allow_low_precision(reason="bf16 routing logits"):
                        nc.vector.tensor_mul(
                            L[:], X_t[:],
                            wb.unsqueeze(1).broadcast_to([128, 9, P, 10]),
                        )
                    nc.scalar.activation(out=L[:], in_=L[:], func=AF.Exp)
                    Z = smp.tile([128, 9, P], F32, tag="Z", name="Zt")
                    nc.vector.tensor_reduce(out=Z[:], in_=L[:],
                                            axis=mybir.AxisListType.X,
                                            op=OP.add)
                    rZf = smp.tile([128, 9, P], F32, tag="rZf", name="rZt")
                    nc.vector.reciprocal_approx_fast(out=rZf[:], in_=Z[:])
                    rZ = smp.tile([128, 9, P], BF16, tag="rZ", name="rZb")
                    with nc.allow_low_precision(reason="bf16 softmax denom"):
                        nc.vector.tensor_copy(out=rZ[:], in_=rZf[:])
                        nc.vector.tensor_mul(L[:], L[:], X_t[:])
                        nc.vector.tensor_mul(
                            L[:], L[:],
                            rZ.unsqueeze(3).broadcast_to([128, 9, P, 10]),
                        )
                    S = pe_sum(L, P, f"S{it}")
                    if not final:
                        v = vrow(S, 1.0, "v1_t", P, np_=1)
                        nc.vector.tensor_add(st["w"][:], st["w"][:], v[:])
                        st["wb"] = bcast_row(st["w"], P, "i2")
                    else:
                        v = vrow(S, 1.0, "v2_t", P, np_=1)
                        fo = sch.tile([1, P, 10], F32, tag="fo_t", name="fo")
                        nc.scalar.activation(out=fo[:], in_=v[:], func=AF.Abs)
                        nc.sync.dma_start(
                            out=out_d[b0:b0 + P, :],
                            in_=fo[0:1, :, :],
                        )
                return cb

            return s0_cb, iter_cb(1), iter_cb(2)

        def chain_sched(u_t, P, b0, sched=None):
            """squash -> x_hat -> 3 routing iterations -> |v| -> out DMA.
            Partition(+j) sums run on PE via ones-matmuls; vrow on a single
            partition. When `sched` (an interleave dict for the NEXT pass's
            prim, plus tap positions) is given, the S/iter stages are
            deferred so the PE-stream ops land at taps where their DVE
            inputs are surely ready. The inline (last-pass) path splits P
            into two half-blocks pipelined across engines: block B's
            DVE-heavy squash/x_hat overlaps block A's iteration stages."""
            if sched is not None:
                X_t = squash_u(u_t, P)
                s0, i1, i2 = chain_stages(X_t, P, b0)
                inter, taps = sched
                inter.setdefault(taps[0], []).append(s0)
                inter.setdefault(taps[1], []).append(i1)
                inter.setdefault(taps[2], []).append(i2)
                return
            X_t = squash_u(u_t, P)
            s0, i1, i2 = chain_stages(X_t, P, b0)
            s0()
            i1()
            i2()

        for _loop in range(loops):
            y_cur = yp.tile([128, 2, 20, 20, passes[0]], BF16, tag="y",
                            name="y_t")
            if _loop == 0:
                # head: xc0 already DMA'd above, then the big resident load
                load_wres()
                xc_cur = xc0_t
            else:
                xc_cur = xcols_load(0)
            # lead-in: emit only the chunk pairs prim tap (0,0) needs
            # (positions up to (10,10) -> flat 210 -> chunk ceil(211*P/512));
            # the rest interleave into the first prim taps.
            cbs0 = conv0_ops_cmajor(y_cur, xc_cur, passes[0])
            need = 2 * ((211 * passes[0] + CHUNK - 1) // CHUNK)
            for cb in cbs0[:need]:
                cb()

            inter = {}
            for ci, cb in enumerate(cbs0[need:]):
                inter.setdefault(1 + ci, []).append(cb)
            for i, P in enumerate(passes):
                nxt = {}
                y_next = None
                if i + 1 < len(passes):
                    Pn = passes[i + 1]
                    y_next = yp.tile([128, 2, 20, 20, Pn], BF16, tag="y",
                                     name="y_t")
                    xc_box = []

                    def load_next(i=i):
                        xc_box.append(xcols_load(i + 1))

                    inter.setdefault(4, []).append(load_next)
                    cbs_box = []

                    def make_cbs(y_next=y_next, Pn=Pn):
                        cbs_box.extend(conv0_ops(y_next, xc_box[0], Pn))

                    inter.setdefault(24, []).append(make_cbs)
                    # spread conv0 chunks over taps 25..72
                    n_cb = 2 * ((400 * Pn + CHUNK - 1) // CHUNK)
                    for ci in range(n_cb):
                        def run_cb(ci=ci):
                            cbs_box[ci]()
                        inter.setdefault(25 + 2 * ci, []).append(run_cb)
                pr = prim_pass(y_cur, P, interleave=inter)
                u_t = stage2_pass(pr, P)
                if i + 1 < len(passes):
                    # defer the chain's PE-stream stages into the next
                    # pass's prim at taps where their inputs are ready
                    taps = (36, 54, 72) if passes[i + 1] == 14 else (56, 70, 79)
                    chain_sched(u_t, P, offs[i], sched=(nxt, taps))
                else:
                    chain_sched(u_t, P, offs[i], sched=None)
                inter = nxt
                y_cur = y_next

    nc.compile()
    return nc


# ---------------- host side ----------------

_CACHE = {}


def _prep(x, conv0_w, conv0_b, prim_w, prim_b, digit_w):
    B = x.shape[0]
    xw = np.lib.stride_tricks.sliding_window_view(x[:, 0], (9, 9), axis=(1, 2))
    # (B, 20, 20, 9, 9) -> (B, 9, 9, 20, 20) -> (B, 81, 400)
    xcols_std = np.ascontiguousarray(
        xw.transpose(0, 3, 4, 1, 2).reshape(B, 81, 400)
    )
    c0wT = np.ascontiguousarray(
        conv0_w.reshape(256, 81).T
    ).astype(ml_dtypes.bfloat16)
    c0b2 = np.ascontiguousarray(
        conv0_b.reshape(2, 128).T, dtype=np.float32
    )
    # prim weights resident layout [p, t, kt, r, q]: co = 2q+r, ci = kt*128+p
    pw = prim_w.reshape(128, 2, 2, 128, 81)  # (q, r, kt, p, t)
    wres = np.ascontiguousarray(
        pw.transpose(3, 4, 2, 1, 0)  # (p, t, kt, r, q)
    ).astype(ml_dtypes.bfloat16)
    pbr = np.ascontiguousarray(prim_b.reshape(128, 2), dtype=np.float32)
    dwr = np.ascontiguousarray(
        digit_w[:, :, 0, :].transpose(1, 0, 2).reshape(128, 9, 10, 8),
        dtype=np.float32,
    )
    return xcols_std, c0wT, c0b2, pbr, wres, dwr


def _xcols_interleave(xcols_core):
    """[Bc, 81, 400] -> [81, 400*Bc] with per-pass (pos, image) interleave."""
    Bc = xcols_core.shape[0]
    passes = PASSES if Bc == B_CORE else None
    if passes is None:
        out, rem = [], Bc
        while rem > 0:
            p = min(14, rem)
            out.append(p)
            rem -= p
        passes = tuple(out)
    blocks = []
    b0 = 0
    for P in passes:
        blk = xcols_core[b0:b0 + P]              # [P, 81, 400]
        blocks.append(blk.transpose(1, 2, 0).reshape(81, 400 * P))
        b0 += P
    return np.ascontiguousarray(
        np.concatenate(blocks, axis=1)
    ).astype(ml_dtypes.bfloat16)


def make_in_maps(x, conv0_w, conv0_b, prim_w, prim_b, digit_w):
    x = np.asarray(x, dtype=np.float32)
    conv0_w = np.asarray(conv0_w, dtype=np.float32)
    conv0_b = np.asarray(conv0_b, dtype=np.float32)
    prim_w = np.asarray(prim_w, dtype=np.float32)
    prim_b = np.asarray(prim_b, dtype=np.float32)
    digit_w = np.asarray(digit_w, dtype=np.float32)

    xcols_std, c0wT, c0b2, pbr, wres, dwr = _prep(
        x, conv0_w, conv0_b, prim_w, prim_b, digit_w
    )
    onesb = np.ones((128, 1), dtype=ml_dtypes.bfloat16)
    onesr = np.ones((1, 128), dtype=ml_dtypes.bfloat16)
    in_maps = []
    for c in range(N_CORES):
        sl = slice(c * B_CORE, (c + 1) * B_CORE)
        in_maps.append(
            {
                "xcols": _xcols_interleave(xcols_std[sl]),
                "c0wT": c0wT,
                "c0b2": c0b2,
                "pbr": pbr,
                "wres": wres,
                "dwr": dwr,
                "onesb": onesb,
                "onesr": onesr,
            }
        )
    return in_maps


def kernel(x, conv0_w, conv0_b, prim_w, prim_b, digit_w):
    from concourse.bass_utils import run_bass_kernel_spmd

    in_maps = make_in_maps(x, conv0_w, conv0_b, prim_w, prim_b, digit_w)

    if "nc" not in _CACHE:
        _CACHE["nc"] = build(B_CORE)
    nc = _CACHE["nc"]

    res = run_bass_kernel_spmd(nc, in_maps, core_ids=list(range(N_CORES)))
    out = np.concatenate([r["out"] for r in res.results], axis=0)
    return out.astype(np.float32)


if __name__ == "__main__":
    # quick smoke build
    nc = build()
    print("build ok")


# revision 37
# speedup vs baseline: 1.2202x; 1.0052x over previous
"""CapsuleNet forward kernel for 8 Trainium2 NeuronCores (pure data parallel).

Host side: im2col + weight-layout prep in numpy; batch 512 sharded 64/core.
Device side (per core), v4 design:
  - y stored P-inner [128, kt, 20, 20, P] so prim-conv rhs slices
    [6, 6, P] have a contiguous inner dim -> full-rate PE streaming
    (216ns/504-col matmul vs 302ns for the P-outer layout). NOTE: prim
    matmul speed is sensitive to the SBUF placement of y/wres — pool
    and tag changes in consts/xcp/yp shift addresses and have caused
    reproducible 216->259ns regressions; re-profile after any change.
  - conv0 computed pass-wide from (pos, image)-interleaved im2col
    columns; PSUM evacuates with contiguous writes into the y layout.
    Pass-0 lead-in emits only the chunks prim tap (0,0) needs; the
    rest interleave into the first prim taps.
  - prim PSUM is (pos-major, image-minor) [128, 36, P]; evacuated by 4
    ACT copies straight into capsule-major u[q, j, b, k]
  - routing chain: partition(+j) sums via PE ones-matmuls, squash rows
    (vrow) on a single partition, broadcast back via outer-product
    ones-matmul. For overlapped passes the S/iter stages defer into
    the NEXT pass's prim tap stream at taps where their DVE inputs are
    ready, so the PE never stalls mid-stream; only the last pass's
    chain is exposed (~45us, DVE-serial).
  - head: c0wT + pass-0 xcols DMA'd before the 10.6MB resident weights
"""

import sys

if "/opt/trn_rl_repo" not in sys.path:
    sys.path.insert(0, "/opt/trn_rl_repo")

from contextlib import ExitStack

import ml_dtypes
import numpy as np

import concourse.bacc as bacc
import concourse.bass as bass
import concourse.bass_isa as bass_isa
import concourse.tile as tile
from concourse import mybir

F32 = mybir.dt.float32
BF16 = mybir.dt.bfloat16
AF = mybir.ActivationFunctionType
OP = mybir.AluOpType
RED = bass_isa.ReduceOp

N_CORES = 8
B_FULL = 512
B_CORE = B_FULL // N_CORES

PASSES = (14, 14, 14, 14, 8)
CHUNK = 512  # conv0 psum chunk columns


def build(B=B_CORE, passes=None, loops=1):
    """Build the Bass module for one core processing B images.

    loops>1 repeats the whole program (same output) for benchmarking."""
    if passes is None:
        passes = PASSES if B == B_CORE else None
    if passes is None:
        out, rem = [], B
        while rem > 0:
            p = min(14, rem)
            out.append(p)
            rem -= p
        passes = tuple(out)
    assert sum(passes) == B
    assert all(p <= 14 for p in passes)

    nc = bacc.Bacc("TRN2")

    # ---- DRAM I/O ----
    xcols_d = nc.dram_tensor("xcols", [81, 400 * B], BF16,
                             kind="ExternalInput")
    c0wT_d = nc.dram_tensor("c0wT", [81, 256], BF16, kind="ExternalInput")
    c0b_d = nc.dram_tensor("c0b2", [128, 2], F32, kind="ExternalInput")
    pbr_d = nc.dram_tensor("pbr", [128, 2], F32, kind="ExternalInput")
    # resident prim weights: [p(=ci%128), tap, kt(=ci//128), r(=co%2), q(=co//2)]
    wres_d = nc.dram_tensor("wres", [128, 81, 2, 2, 128], BF16,
                            kind="ExternalInput")
    # capsule weights: [m(=i//9), j(=i%9), o, k]
    dwr_d = nc.dram_tensor("dwr", [128, 9, 10, 8], F32, kind="ExternalInput")
    onesb_d = nc.dram_tensor("onesb", [128, 1], BF16, kind="ExternalInput")
    onesr_d = nc.dram_tensor("onesr", [1, 128], BF16, kind="ExternalInput")
    out_d = nc.dram_tensor("out", [B, 10], F32, kind="ExternalOutput")

    with ExitStack() as ctx:
        tc = ctx.enter_context(tile.TileContext(nc))

        consts = ctx.enter_context(tc.tile_pool(name="consts", bufs=1))
        xcp = ctx.enter_context(tc.tile_pool(name="xcp", bufs=2))
        yp = ctx.enter_context(tc.tile_pool(name="yp", bufs=2))
        up = ctx.enter_context(tc.tile_pool(name="up", bufs=2))
        xhp = ctx.enter_context(tc.tile_pool(name="xhp", bufs=2))
        tmpp = ctx.enter_context(tc.tile_pool(name="tmpp", bufs=1))
        smp = ctx.enter_context(tc.tile_pool(name="smp", bufs=1))
        sch = ctx.enter_context(tc.tile_pool(name="sch", bufs=1))
        pc0 = ctx.enter_context(tc.tile_pool(name="pc0", bufs=3, space="PSUM"))
        ppr = ctx.enter_context(tc.tile_pool(name="ppr", bufs=2, space="PSUM"))
        pch = ctx.enter_context(tc.tile_pool(name="pch", bufs=1, space="PSUM"))

        offs = []
        b0 = 0
        for P in passes:
            offs.append(b0)
            b0 += P

        # ---- head-critical DMAs first: conv0 needs c0wT + early xcols ----
        c0wT_t = consts.tile([81, 256], BF16)
        nc.sync.dma_start(out=c0wT_t, in_=c0wT_d[:, :])
        xc0_t = xcp.tile([81, 400 * passes[0]], BF16, tag="xc", name="xc_t")
        nc.sync.dma_start(out=xc0_t, in_=xcols_d[:, 0:400 * passes[0]])
        c0b_t = consts.tile([128, 2], F32)
        nc.sync.dma_start(out=c0b_t, in_=c0b_d[:, :])
        pb_t = consts.tile([128, 2], F32)
        nc.sync.dma_start(out=pb_t, in_=pbr_d[:, :])
        dwr_t = consts.tile([128, 9, 10, 8], F32)
        nc.sync.dma_start(out=dwr_t, in_=dwr_d[:, :, :, :])
        onesb_t = consts.tile([128, 1], BF16)
        nc.sync.dma_start(out=onesb_t, in_=onesb_d[:, :])
        onesr_t = consts.tile([1, 128], BF16)
        nc.sync.dma_start(out=onesr_t, in_=onesr_d[:, :])
        dwrb_t = consts.tile([128, 9, 10, 8], BF16)
        nc.vector.tensor_copy(out=dwrb_t[:], in_=dwr_t[:])

        def xcols_load(i):
            P = passes[i]
            xc_t = xcp.tile([81, 400 * P], BF16, tag="xc", name="xc_t")
            o = offs[i] * 400
            nc.sync.dma_start(out=xc_t, in_=xcols_d[:, o:o + 400 * P])
            return xc_t

        _CB = {"n": 0}

        def conv0_ops(y_t, xc_t, P, segs=None):
            """Pass-wide conv0: (pos,img)-interleaved columns; returns list
            of callbacks each doing one chunk matmul+relu-evac. segs: list
            of (tile, col_start, col_end) overriding the single xc_t."""
            total = 400 * P
            yf = y_t.rearrange("p t h w b -> p t (h w b)")
            if segs is None:
                segs = [(xc_t, 0, total)]

            def rhs_for(c0, cw):
                for t_, s_, e_ in segs:
                    if c0 >= s_ and c0 + cw <= e_:
                        return t_[:, c0 - s_:c0 - s_ + cw]
                raise AssertionError("chunk crosses segment boundary")

            cbs = []
            for mt in range(2):
                for c0 in range(0, total, CHUNK):
                    cw = min(CHUNK, total - c0)

                    def cb(mt=mt, c0=c0, cw=cw):
                        ps = pc0.tile([128, CHUNK], F32, tag="pc0",
                                      name="c0ps")
                        nc.tensor.matmul(
                            out=ps[:, 0:cw],
                            lhsT=c0wT_t[:, mt * 128:(mt + 1) * 128],
                            rhs=rhs_for(c0, cw),
                            start=True, stop=True,
                        )
                        nc.scalar.activation(
                            out=yf[:, mt, c0:c0 + cw], in_=ps[:, 0:cw],
                            func=AF.Relu, bias=c0b_t[:, mt:mt + 1],
                            scale=1.0,
                        )
                    cbs.append(cb)
            return cbs

        def conv0_ops_cmajor(y_t, xc_t, P):
            """conv0 chunk callbacks ordered chunk-major (mt pairs), so the
            low-position chunks needed by early prim taps evacuate first."""
            cbs = conv0_ops(y_t, xc_t, P)
            n = len(cbs) // 2
            out = []
            for c in range(n):
                out.append(cbs[c])
                out.append(cbs[n + c])
            return out

        wres_ts = []

        def load_wres():
            # per-chunk weight tiles so early prim taps don't wait on the
            # full 10.6MB resident load
            for t0 in range(0, 81, 9):
                wt = consts.tile([128, 9, 2, 2, 128], BF16,
                                 name=f"wres{t0}")
                nc.sync.dma_start(out=wt, in_=wres_d[:, t0:t0 + 9])
                wres_ts.append(wt)

        def prim_pass(y_t, P, interleave=None):
            """prim caps conv: accumulate 81 taps x 2kt into 2 r-PSUM tiles
            shaped [128, 6, 6, P] (pos-major, image-minor)."""
            pr = [ppr.tile([128, 6, 6, P], F32, tag=f"ppr{r}",
                           name=f"ppr_{r}") for r in range(2)]
            for t in range(81):
                if interleave and t in interleave:
                    for cb in interleave[t]:
                        cb()
                kh, kw = t // 9, t % 9
                wt = wres_ts[t // 9]
                for kt in range(2):
                    for r in range(2):
                        rhs = y_t[:, kt, kh:kh + 12:2, kw:kw + 12:2, :]
                        nc.tensor.matmul(
                            out=pr[r].rearrange("p h w b -> p (h w b)"),
                            lhsT=wt[:, t % 9, kt, r, :],
                            rhs=rhs,
                            start=(t == 0 and kt == 0),
                            stop=(t == 80 and kt == 1),
                        )
            return pr

        def stage2_pass(pr, P):
            """evacuate prim PSUM (+bias) into capsule-major u[q, j, b, k].
            PSUM columns are (pos, b); 36r+pos = 8j+k."""
            u_t = up.tile([128, 9, P, 8], F32, tag="u", name="u_t")
            V0 = pr[0].rearrange("p h w b -> p (h w) b")
            V1 = pr[1].rearrange("p h w b -> p (h w) b")
            nc.scalar.activation(
                out=u_t[:, 0:4, :, :],
                in_=V0[:, 0:32, :].rearrange("p (j k) b -> p j k b", k=8)
                .transpose([0, 1, 3, 2]),
                func=AF.Identity, bias=pb_t[:, 0:1], scale=1.0,
            )
            nc.scalar.activation(
                out=u_t[:, 4, :, 0:4],
                in_=V0[:, 32:36, :].transpose([0, 2, 1]),
                func=AF.Identity, bias=pb_t[:, 0:1], scale=1.0,
            )
            nc.scalar.activation(
                out=u_t[:, 4, :, 4:8],
                in_=V1[:, 0:4, :].transpose([0, 2, 1]),
                func=AF.Identity, bias=pb_t[:, 1:2], scale=1.0,
            )
            nc.scalar.activation(
                out=u_t[:, 5:9, :, :],
                in_=V1[:, 4:36, :].rearrange("p (j k) b -> p j k b", k=8)
                .transpose([0, 1, 3, 2]),
                func=AF.Identity, bias=pb_t[:, 1:2], scale=1.0,
            )
            return u_t

        def vrow(S_t, alpha, tag, P, np_=128):
            """squash scalar per (b,o): v = s^3/((1+s^2)(|s|+eps)), s=alpha*S.
            Works on [np_, P, 10] tiles (np_=128 replicated, or 1)."""
            sh = [np_, P, 10]
            pool = smp if np_ == 128 else sch
            # DVE-only: same-engine deps avoid semaphore hops and keep
            # Square/Abs out of the ACT tables (no ACT_TABLE_LOAD swaps)
            if alpha != 1.0:
                ts_ = pool.tile(sh, F32, tag="vr_ts", name="vr_ts")
                nc.vector.tensor_scalar_mul(ts_[:], S_t[:], alpha)
            else:
                ts_ = S_t
            s2 = pool.tile(sh, F32, tag="vr_s2", name="vr_s2")
            nc.vector.tensor_mul(s2[:], ts_[:], ts_[:])
            ab = pool.tile(sh, F32, tag="vr_ab", name="vr_ab")
            nc.vector.scalar_tensor_tensor(
                out=ab[:], in0=ts_[:], scalar=-1.0, in1=ts_[:],
                op0=OP.mult, op1=OP.max,
            )
            nc.vector.scalar_tensor_tensor(
                out=ab[:], in0=s2[:], scalar=1.0, in1=ab[:],
                op0=OP.add, op1=OP.mult,
            )
            nc.vector.tensor_scalar_add(ab[:], ab[:], 1e-8)
            nc.vector.reciprocal_approx_fast(out=ab[:], in_=ab[:])
            nc.vector.tensor_mul(s2[:], ts_[:], s2[:])
            v = pool.tile(sh, F32, tag=tag, name=tag)
            nc.vector.tensor_mul(v[:], s2[:], ab[:])
            return v

        def squash_u(u_t, P):
            """in-place squash of u over k plus bf16 copy + x_hat."""
            usq = tmpp.tile([128, 9, P, 8], F32, tag="usq", name="usq")
            nc.vector.tensor_mul(usq[:], u_t[:], u_t[:])
            n2 = smp.tile([128, 9, P], F32, tag="n2", name="n2")
            nc.vector.tensor_reduce(out=n2[:], in_=usq[:],
                                    axis=mybir.AxisListType.X, op=OP.add)
            nrm = smp.tile([128, 9, P], F32, tag="nrm", name="nrm")
            nc.scalar.activation(out=nrm[:], in_=n2[:], func=AF.Sqrt)
            nc.vector.tensor_scalar_add(n2[:], n2[:], 1.0)
            nc.vector.reciprocal_approx_fast(out=n2[:], in_=n2[:])
            nc.vector.tensor_mul(nrm[:], nrm[:], n2[:])
            ub = tmpp.tile([128, 9, P, 8], BF16, tag="ub", name="ub")
            with nc.allow_low_precision(reason="bf16 squashed u"):
                nc.vector.tensor_mul(
                    ub[:], u_t[:],
                    nrm.unsqueeze(3).broadcast_to([128, 9, P, 8]),
                )

            X_t = xhp.tile([128, 9, P, 10], BF16, tag="X", name="X_t")
            with nc.allow_low_precision(reason="bf16 x_hat (8-term dots)"):
                for o in range(10):
                    xt = tmpp.tile([128, 9, P, 8], BF16, tag="xh",
                                   name="xh_tmp")
                    nc.vector.tensor_mul(
                        xt[:], ub[:],
                        dwrb_t[:, :, o, :].unsqueeze(2)
                        .broadcast_to([128, 9, P, 8]),
                    )
                    nc.vector.tensor_reduce(out=X_t[:, :, :, o], in_=xt[:],
                                            axis=mybir.AxisListType.X,
                                            op=OP.add)
            return X_t

        def pe_sum(src_t, P, name, scale01=False):
            """S[1, P*10] = sum over partitions and j of src[128, 9, P, 10]
            via 9 accumulating ones-matmuls; evac to a [1, P, 10] f32 tile.
            scale01: use the 0.1-valued lhsT column (uniform softmax S0)."""
            col = 1 if scale01 else 0
            ps = pch.tile([1, P * 10], F32, tag="pchS", name=name + "p")
            for j in range(9):
                nc.tensor.matmul(
                    out=ps[:, :],
                    lhsT=onesb_t[:, col:col + 1],
                    rhs=src_t[:, j].rearrange("p b o -> p (b o)"),
                    start=(j == 0), stop=(j == 8),
                )
            Sr = sch.tile([1, P, 10], F32, tag="Sr_" + name, name=name)
            nc.vector.tensor_copy(out=Sr.rearrange("p b o -> p (b o)"),
                                  in_=ps[:, :])
            return Sr

        def bcast_row(vrow_t, P, name):
            """broadcast [1, P, 10] f32 -> [128, P, 10] bf16 via an
            outer-product ones-matmul (PE) + ACT evac."""
            vb = sch.tile([1, P, 10], BF16, tag="vb_" + name, name="vb")
            nc.vector.tensor_copy(out=vb[:], in_=vrow_t[:])
            ps = pch.tile([128, P * 10], F32, tag="pchS", name="bc_" + name)
            nc.tensor.matmul(
                out=ps[:, :],
                lhsT=onesr_t[:, :],
                rhs=vb.rearrange("p b o -> p (b o)"),
                start=True, stop=True,
            )
            wb = smp.tile([128, P, 10], BF16, tag="wbt", name="wb_" + name)
            nc.vector.tensor_copy(out=wb.rearrange("p b o -> p (b o)"),
                                  in_=ps[:, :])
            return wb

        def chain_stages(X_t, P, b0):
            """Make the S0/iter1/iter2 stage callbacks for one X block."""
            st = {}

            def s0_cb():
                S0 = pe_sum(X_t, P, "S0")
                st["w"] = vrow(S0, 0.1, "w_acc_t", P, np_=1)
                st["wb"] = bcast_row(st["w"], P, "i1")

            def iter_cb(it):
                final = it == 2

                def cb():
                    wb = st["wb"]
                    L = tmpp.tile([128, 9, P, 10], BF16, tag="L", name="Lt")
                    with nc.allow_low_precision(reason="bf16 routing logits"):
                        nc.vector.tensor_mul(
                            L[:], X_t[:],
                            wb.unsqueeze(1).broadcast_to([128, 9, P, 10]),
                        )
                    nc.scalar.activation(out=L[:], in_=L[:], func=AF.Exp)
                    Z = smp.tile([128, 9, P], F32, tag="Z", name="Zt")
                    nc.vector.tensor_reduce(out=Z[:], in_=L[:],
                                            axis=mybir.AxisListType.X,
                                            op=OP.add)
                    rZf = smp.tile([128, 9, P], F32, tag="rZf", name="rZt")
                    nc.vector.reciprocal_approx_fast(out=rZf[:], in_=Z[:])
                    rZ = smp.tile([128, 9, P], BF16, tag="rZ", name="rZb")
                    with nc.allow_low_precision(reason="bf16 softmax denom"):
                        nc.vector.tensor_copy(out=rZ[:], in_=rZf[:])
                        nc.vector.tensor_mul(L[:], L[:], X_t[:])
                        nc.vector.tensor_mul(
                            L[:], L[:],
                            rZ.unsqueeze(3).broadcast_to([128, 9, P, 10]),
                        )
                    S = pe_sum(L, P, f"S{it}")
                    if not final:
                        v = vrow(S, 1.0, "v1_t", P, np_=1)
                        nc.vector.tensor_add(st["w"][:], st["w"][:], v[:])
                        st["wb"] = bcast_row(st["w"], P, "i2")
                    else:
                        v = vrow(S, 1.0, "v2_t", P, np_=1)
                        fo = sch.tile([1, P, 10], F32, tag="fo_t", name="fo")
                        nc.vector.scalar_tensor_tensor(
                            out=fo[:], in0=v[:], scalar=-1.0, in1=v[:],
                            op0=OP.mult, op1=OP.max,
                        )
                        nc.sync.dma_start(
                            out=out_d[b0:b0 + P, :],
                            in_=fo[0:1, :, :],
                        )
                return cb

            return s0_cb, iter_cb(1), iter_cb(2)

        def chain_sched(u_t, P, b0, sched=None):
            """squash -> x_hat -> 3 routing iterations -> |v| -> out DMA.
            Partition(+j) sums run on PE via ones-matmuls; vrow on a single
            partition. When `sched` (an interleave dict for the NEXT pass's
            prim, plus tap positions) is given, the S/iter stages are
            deferred so the PE-stream ops land at taps where their DVE
            inputs are surely ready. The inline (last-pass) path splits P
            into two half-blocks pipelined across engines: block B's
            DVE-heavy squash/x_hat overlaps block A's iteration stages."""
            if sched is not None:
                X_t = squash_u(u_t, P)
                s0, i1, i2 = chain_stages(X_t, P, b0)
                inter, taps = sched
                inter.setdefault(taps[0], []).append(s0)
                inter.setdefault(taps[1], []).append(i1)
                inter.setdefault(taps[2], []).append(i2)
                return
            X_t = squash_u(u_t, P)
            s0, i1, i2 = chain_stages(X_t, P, b0)
            s0()
            i1()
            i2()

        for _loop in range(loops):
            y_cur = yp.tile([128, 2, 20, 20, passes[0]], BF16, tag="y",
                            name="y_t")
            if _loop == 0:
                # head: xc0 already DMA'd above, then the big resident load
                load_wres()
                xc_cur = xc0_t
            else:
                xc_cur = xcols_load(0)
            # lead-in: emit only the chunk pairs prim tap (0,0) needs
            # (positions up to (10,10) -> flat 210 -> chunk ceil(211*P/512));
            # the rest interleave into the first prim taps.
            cbs0 = conv0_ops_cmajor(y_cur, xc_cur, passes[0])
            need = 2 * ((211 * passes[0] + CHUNK - 1) // CHUNK)
            for cb in cbs0[:need]:
                cb()

            inter = {}
            for ci, cb in enumerate(cbs0[need:]):
                inter.setdefault(1 + ci, []).append(cb)
            for i, P in enumerate(passes):
                nxt = {}
                y_next = None
                if i + 1 < len(passes):
                    Pn = passes[i + 1]
                    y_next = yp.tile([128, 2, 20, 20, Pn], BF16, tag="y",
                                     name="y_t")
                    xc_box = []

                    def load_next(i=i):
                        xc_box.append(xcols_load(i + 1))

                    inter.setdefault(4, []).append(load_next)
                    cbs_box = []

                    def make_cbs(y_next=y_next, Pn=Pn):
                        cbs_box.extend(conv0_ops(y_next, xc_box[0], Pn))

                    inter.setdefault(24, []).append(make_cbs)
                    # spread conv0 chunks over taps 25..72
                    n_cb = 2 * ((400 * Pn + CHUNK - 1) // CHUNK)
                    for ci in range(n_cb):
                        def run_cb(ci=ci):
                            cbs_box[ci]()
                        inter.setdefault(25 + 2 * ci, []).append(run_cb)
                pr = prim_pass(y_cur, P, interleave=inter)
                u_t = stage2_pass(pr, P)
                if i + 1 < len(passes):
                    # defer the chain's PE-stream stages into the next
                    # pass's prim at taps where their inputs are ready
                    taps = (36, 54, 72) if passes[i + 1] == 14 else (56, 70, 79)
                    chain_sched(u_t, P, offs[i], sched=(nxt, taps))
                else:
                    chain_sched(u_t, P, offs[i], sched=None)
                inter = nxt
                y_cur = y_next

    nc.compile()
    return nc


# ---------------- host side ----------------

_CACHE = {}


def _prep(x, conv0_w, conv0_b, prim_w, prim_b, digit_w):
    B = x.shape[0]
    xw = np.lib.stride_tricks.sliding_window_view(x[:, 0], (9, 9), axis=(1, 2))
    # (B, 20, 20, 9, 9) -> (B, 9, 9, 20, 20) -> (B, 81, 400)
    xcols_std = np.ascontiguousarray(
        xw.transpose(0, 3, 4, 1, 2).reshape(B, 81, 400)
    )
    c0wT = np.ascontiguousarray(
        conv0_w.reshape(256, 81).T
    ).astype(ml_dtypes.bfloat16)
    c0b2 = np.ascontiguousarray(
        conv0_b.reshape(2, 128).T, dtype=np.float32
    )
    # prim weights resident layout [p, t, kt, r, q]: co = 2q+r, ci = kt*128+p
    pw = prim_w.reshape(128, 2, 2, 128, 81)  # (q, r, kt, p, t)
    wres = np.ascontiguousarray(
        pw.transpose(3, 4, 2, 1, 0)  # (p, t, kt, r, q)
    ).astype(ml_dtypes.bfloat16)
    pbr = np.ascontiguousarray(prim_b.reshape(128, 2), dtype=np.float32)
    dwr = np.ascontiguousarray(
        digit_w[:, :, 0, :].transpose(1, 0, 2).reshape(128, 9, 10, 8),
        dtype=np.float32,
    )
    return xcols_std, c0wT, c0b2, pbr, wres, dwr


def _xcols_interleave(xcols_core):
    """[Bc, 81, 400] -> [81, 400*Bc] with per-pass (pos, image) interleave."""
    Bc = xcols_core.shape[0]
    passes = PASSES if Bc == B_CORE else None
    if passes is None:
        out, rem = [], Bc
        while rem > 0:
            p = min(14, rem)
            out.append(p)
            rem -= p
        passes = tuple(out)
    blocks = []
    b0 = 0
    for P in passes:
        blk = xcols_core[b0:b0 + P]              # [P, 81, 400]
        blocks.append(blk.transpose(1, 2, 0).reshape(81, 400 * P))
        b0 += P
    return np.ascontiguousarray(
        np.concatenate(blocks, axis=1)
    ).astype(ml_dtypes.bfloat16)


def make_in_maps(x, conv0_w, conv0_b, prim_w, prim_b, digit_w):
    x = np.asarray(x, dtype=np.float32)
    conv0_w = np.asarray(conv0_w, dtype=np.float32)
    conv0_b = np.asarray(conv0_b, dtype=np.float32)
    prim_w = np.asarray(prim_w, dtype=np.float32)
    prim_b = np.asarray(prim_b, dtype=np.float32)
    digit_w = np.asarray(digit_w, dtype=np.float32)

    xcols_std, c0wT, c0b2, pbr, wres, dwr = _prep(
        x, conv0_w, conv0_b, prim_w, prim_b, digit_w
    )
    onesb = np.ones((128, 1), dtype=ml_dtypes.bfloat16)
    onesr = np.ones((1, 128), dtype=ml_dtypes.bfloat16)
    in_maps = []
    for c in range(N_CORES):
        sl = slice(c * B_CORE, (c + 1) * B_CORE)
        in_maps.append(
            {
                "xcols": _xcols_interleave(xcols_std[sl]),
                "c0wT": c0wT,
                "c0b2": c0b2,
                "pbr": pbr,
                "wres": wres,
                "dwr": dwr,
                "onesb": onesb,
                "onesr": onesr,
            }
        )
    return in_maps


def kernel(x, conv0_w, conv0_b, prim_w, prim_b, digit_w):
    from concourse.bass_utils import run_bass_kernel_spmd

    in_maps = make_in_maps(x, conv0_w, conv0_b, prim_w, prim_b, digit_w)

    if "nc" not in _CACHE:
        _CACHE["nc"] = build(B_CORE)
    nc = _CACHE["nc"]

    res = run_bass_kernel_spmd(nc, in_maps, core_ids=list(range(N_CORES)))
    out = np.concatenate([r["out"] for r in res.results], axis=0)
    return out.astype(np.float32)


if __name__ == "__main__":
    # quick smoke build
    nc = build()
    print("build ok")
